# revision 5
# baseline (speedup 1.0000x reference)
"""Trainium2 Bass kernel for nn_DeepSetAttentionModel (segment_reduce) — v2.

Strategy (pure data parallel, 8 NeuronCores, host featurization):
- Host sorts the 64 set rows by length (desc); rank k -> core k%8, slot
  k//8. All cores run ONE SPMD program whose per-slot chunk counts are
  ceil(slot-max-length/128), so the program is identical across cores.
- Host precomputes the full 34-feature token matrix in feature-major
  (bf16): rows 0:16 sin/cos positional encodings, 16 value, 17:32
  one-hot, 32 const-1 (bias carrier), 33 invalid-flag. It is DMA'd
  straight into the feature-major xT layout the MLP matmuls consume:
  no on-device featurization, no x transposes, no Act-engine Sin.
- Layer-1 biases are folded into the weights via the const-1 row; the
  phi1/psi1 PSUM banks are adjacent so ONE Act relu covers both.
- psi of invalid tokens is forced to a known constant c3: psi_w1 gets
  an extra -1e9 row against the invalid flag (p1=0), and the resulting
  constant bias chain c3 = relu(psi_w3^T relu(psi_b2) + psi_b3) is
  removed with a host-known per-row count correction.
- preattn is computed token-major per 128-chunk ([34,128] stationary
  slices of xT against a [34,4] folded key vector whose row 32 carries
  the agg contribution and row 33 carries -1e9 for invalid tokens).
  |preattn| < 1 for this model, so exp needs no max-subtraction;
  invalid tokens underflow to exactly 0 and drop out of softmax sums.
- enc (phi output) is computed feature-major and moved token-major via
  xbar DMA transposes; the per-chunk weighted head sum contracts a
  129th all-ones column to produce the softmax normalizer for free.
"""
import numpy as np

B, T = 64, 4096
CH = 128
NPOS, V, NMOD = 16, 1, 15
NF = 34                      # 16 tt + 1 value + 15 onehot + 1 const + 1 inv
DP, H = 64, 4
MAXTS = 100.0
NCORES = 8
NSLOTS = B // NCORES


def _build_nc(Cs, tile_mod, bass, mybir):
    """Build the SPMD program for per-slot chunk counts Cs (len 8)."""
    f32 = mybir.dt.float32
    bf16 = mybir.dt.bfloat16
    Alu = mybir.AluOpType
    Act = mybir.ActivationFunctionType
    Cmax = max(Cs)
    NGmax = (((Cmax + 1) * CH) + 511) // 512

    nc = bass.Bass()
    dt_in = {}

    def din(name, shape, dtype=f32):
        dt_in[name] = nc.dram_tensor(name, list(shape), dtype, kind="ExternalInput")
        return dt_in[name]

    # per-core inputs
    d_xfeat = din("xfeat_r", [NSLOTS, NF, T], bf16)
    d_demo = din("demo_r", [NSLOTS, 8], bf16)
    din("ninv_neg", [128, NSLOTS])
    din("recipL1", [128, NSLOTS])
    # replicated constants / weights
    din("ident", [128, 128], bf16)
    WB = [("w1p", [NF, 128], bf16), ("w1s", [NF, 128], bf16),
          ("phi_w2", [128, 128], bf16), ("psi_w2", [128, 128], bf16),
          ("phi_w3", [128, 128], bf16), ("psi_w3", [128, 128], bf16),
          ("rho_attn_w", [128, 128], bf16), ("Vagg", [128, H], bf16),
          ("vxe_const", [NF, H], bf16),
          ("demo_w1", [8, 128], bf16), ("demo_w2", [128, 32], bf16),
          ("rho_w1", [128, H, 128], bf16), ("rho_w2", [128, 128], bf16),
          ("rho_w3", [128, 1], bf16)]
    for nm, shp, dt_ in WB:
        din(nm, shp, dt_)
    BIASES = [("phi_b2", 128), ("psi_b2", 128), ("phi_b3", 128), ("psi_b3", 128),
              ("rho_attn_b", 128), ("demo_b1", 128), ("demo_b2", 32),
              ("rho_b1", 128), ("rho_b2", 128), ("rho_b3", 1)]
    for nm, n in BIASES:
        din(nm, [n])
    d_out = nc.dram_tensor("out", [NSLOTS, 1], f32, kind="ExternalOutput")

    from contextlib import ExitStack
    with tile_mod.TileContext(nc) as tc, ExitStack() as stack:
        cp = stack.enter_context(tc.tile_pool(name="const", bufs=1))
        sp = stack.enter_context(tc.tile_pool(name="sbuf", bufs=1))
        pp = stack.enter_context(tc.tile_pool(name="psum", bufs=1, space="PSUM"))

        # ---- load constants / weights ----
        def load(name, shape, dtype=f32, ap=None):
            t = cp.tile(shape, dtype, tag=name, name=name)
            nc.sync.dma_start(out=t[:], in_=ap if ap is not None else dt_in[name][:])
            return t

        wt = {}
        for nm, shp, dt_ in WB:
            wt[nm] = load(nm, shp, dt_)
        for nm, n in BIASES:
            wt[nm] = load(nm, [n, 1], ap=dt_in[nm][:].unsqueeze(1))
        ident = load("ident", [128, 128], bf16)
        ninv_neg = load("ninv_neg", [128, NSLOTS])
        recipL1 = load("recipL1", [128, NSLOTS])
        demoT = load("demoT", [8, NSLOTS], bf16, ap=d_demo[:].rearrange("r f -> f r"))

        # ---- setup: demo encoder for all 8 slots -> demo_encT [34, 8] ----
        ps_d = pp.tile([128, 512], f32, tag="L2", bufs=2, name="ps_d")
        nc.tensor.matmul(ps_d[:, 0:NSLOTS], wt["demo_w1"][:], demoT[:])
        dh1 = sp.tile([128, NSLOTS], bf16, tag="dh1", name="dh1")
        nc.scalar.activation(dh1[:], ps_d[:, 0:NSLOTS], Act.Relu, bias=wt["demo_b1"][:])
        ps_d2 = pp.tile([128, 512], f32, tag="L2", bufs=2, name="ps_d2")
        nc.tensor.matmul(ps_d2[0:32, 0:NSLOTS], wt["demo_w2"][:], dh1[:])
        demo_encT = cp.tile([NF, NSLOTS], bf16, tag="demo_encT", name="demo_encT")
        nc.scalar.activation(demo_encT[0:32, :], ps_d2[0:32, 0:NSLOTS],
                             Act.Identity, bias=wt["demo_b2"][:])
        nc.gpsimd.memset(demo_encT[32:34, :], 0.0)
        nc.gpsimd.memset(demo_encT[32:33, :], 1.0)

        # ---- setup: c3 = relu(psi_w3^T relu(psi_b2) + psi_b3), correction ----
        p2c = sp.tile([128, 1], bf16, tag="p2c", name="p2c")
        nc.scalar.activation(p2c[:], wt["psi_b2"][:], Act.Relu)
        ps_c3 = pp.tile([128, 512], f32, tag="L2", bufs=2, name="ps_c3")
        nc.tensor.matmul(ps_c3[:, 0:1], wt["psi_w3"][:], p2c[:])
        c3 = sp.tile([128, 1], f32, tag="c3", name="c3")
        nc.scalar.activation(c3[:], ps_c3[:, 0:1], Act.Relu, bias=wt["psi_b3"][:])
        negc3nr = cp.tile([128, NSLOTS], f32, tag="negc3nr", name="negc3nr")
        nc.vector.tensor_tensor(out=negc3nr[:], in0=c3[:].to_broadcast([128, NSLOTS]),
                                in1=ninv_neg[:], op=Alu.mult)

        feat_all = sp.tile([128, NSLOTS, H], bf16, tag="feat_all", name="feat_all")

        # ---- per-row phases (emitted software-pipelined below) ----
        state = {}

        def row_geom(r):
            C = Cs[r]
            return C, C * CH, (C + 1) * CH, ((C + 1) * CH + 511) // 512

        def phase_dma(r):
            C, Tp, Text, NG = row_geom(r)
            xT = sp.tile([NF, (Cmax + 1) * CH], bf16, tag="xT", bufs=2, name="xT")
            nc.sync.dma_start(out=xT[:, 0:Tp], in_=d_xfeat[r, :, 0:Tp])
            nc.gpsimd.memset(xT[:, Tp + 1:Text], 0.0)
            # rows 32/33 (const, inv) both 1.0 in the padding: the -1e9 inv
            # weight dominates, so a stray aggdot via the const row is inert
            nc.gpsimd.memset(xT[32:34, Tp + 1:Text], 1.0)
            nc.vector.tensor_copy(xT[:, Tp:Tp + 1], demo_encT[:, r:r + 1])
            state[r, "xT"] = xT

        def phase_l1(r):
            C, Tp, Text, NG = row_geom(r)
            xT = state[r, "xT"]
            h1p1 = sp.tile([128, NGmax * 1024], bf16, tag="h1p1", bufs=2, name="h1p1")
            for g in range(NG):
                N = min(512, Text - g * 512)
                ps = pp.tile([128, 1024], f32, tag="L1", bufs=2, name="ps_l1")
                nc.tensor.matmul(ps[:, 0:N], wt["w1p"][:], xT[:, g * 512:g * 512 + N])
                nc.tensor.matmul(ps[:, 512:512 + N], wt["w1s"][:], xT[:, g * 512:g * 512 + N])
                if N == 512:
                    nc.scalar.activation(h1p1[:, g * 1024:(g + 1) * 1024], ps[:], Act.Relu)
                else:
                    nc.scalar.activation(h1p1[:, g * 1024:g * 1024 + N], ps[:, 0:N], Act.Relu)
                    nc.scalar.activation(h1p1[:, g * 1024 + 512:g * 1024 + 512 + N],
                                         ps[:, 512:512 + N], Act.Relu)
            state[r, "h1p1"] = h1p1

        def phase_psi23(r):
            C, Tp, Text, NG = row_geom(r)
            h1p1 = state[r, "h1p1"]
            p2 = sp.tile([128, (Cmax + 1) * CH], bf16, tag="p2", bufs=2, name="p2")
            for g in range(NG):
                N = min(512, Text - g * 512)
                ps = pp.tile([128, 512], f32, tag="L2", bufs=2, name="ps_psi2")
                nc.tensor.matmul(ps[:, 0:N], wt["psi_w2"][:],
                                 h1p1[:, g * 1024 + 512:g * 1024 + 512 + N])
                nc.vector.tensor_scalar(p2[:, g * 512:g * 512 + N], ps[:, 0:N],
                                        wt["psi_b2"][:], 0.0, Alu.add, Alu.max)
            acc = sp.tile([128, 16], f32, tag="acc", bufs=2, name="acc")
            psi3s = sp.tile([128, 512], bf16, tag="psi3s", bufs=2, name="psi3s")
            for g in range(NG):
                N = min(512, Text - g * 512)
                ps = pp.tile([128, 512], f32, tag="L2", bufs=2, name="ps_psi3")
                nc.tensor.matmul(ps[:, 0:N], wt["psi_w3"][:], p2[:, g * 512:g * 512 + N])
                nc.scalar.activation(psi3s[:, 0:N], ps[:, 0:N], Act.Relu,
                                     bias=wt["psi_b3"][:], accum_out=acc[:, g:g + 1])
            # agg DVE part (runs while PE continues with phi2/enc3)
            aggs = sp.tile([128, 1], f32, tag="aggs", bufs=2, name="aggs")
            nc.vector.tensor_reduce(aggs[:], acc[:, 0:NG], mybir.AxisListType.X, Alu.add)
            agg_in = sp.tile([128, 1], bf16, tag="aggin", bufs=2, name="agg_in")
            nc.vector.tensor_scalar(agg_in[:], aggs[:], negc3nr[:, r:r + 1],
                                    recipL1[:, r:r + 1], Alu.add, Alu.mult)
            state[r, "agg_in"] = agg_in

        def phase_phi2(r):
            C, Tp, Text, NG = row_geom(r)
            h1p1 = state[r, "h1p1"]
            h2 = sp.tile([128, (Cmax + 1) * CH], bf16, tag="h2", bufs=2, name="h2")
            for g in range(NG):
                N = min(512, Text - g * 512)
                ps = pp.tile([128, 512], f32, tag="L2", bufs=2, name="ps_phi2")
                nc.tensor.matmul(ps[:, 0:N], wt["phi_w2"][:], h1p1[:, g * 1024:g * 1024 + N])
                nc.vector.tensor_scalar(h2[:, g * 512:g * 512 + N], ps[:, 0:N],
                                        wt["phi_b2"][:], 0.0, Alu.add, Alu.max)
            state[r, "h2"] = h2

        def phase_enc3(r):
            C, Tp, Text, NG = row_geom(r)
            h2 = state[r, "h2"]
            enc_tok = sp.tile([128, Cmax + 1, 144], bf16, tag="enctok", bufs=2, name="enc_tok")
            nc.gpsimd.memset(enc_tok[:, 0:C + 1, 128:129], 1.0)
            for g in range(NG):
                N = min(512, Text - g * 512)
                ps = pp.tile([128, 512], f32, tag="L2", bufs=2, name="ps_phi3")
                nc.tensor.matmul(ps[:, 0:N], wt["phi_w3"][:], h2[:, g * 512:g * 512 + N])
                encfm = sp.tile([128, 512], bf16, tag="encfm", bufs=2, name="encfm")
                nc.vector.tensor_scalar(encfm[:, 0:N], ps[:, 0:N],
                                        wt["phi_b3"][:], 0.0, Alu.add, Alu.max)
                c0 = g * 4
                nch = min(4, C + 1 - c0)
                nc.sync.dma_start_transpose(out=enc_tok[:, c0:c0 + nch, 0:128],
                                            in_=encfm[:, 0:nch * CH])
            state[r, "enc_tok"] = enc_tok

        def phase_agg_pe(r):
            ps_a = pp.tile([128, 512], f32, tag="L2", bufs=2, name="ps_a")
            nc.tensor.matmul(ps_a[:, 0:1], wt["rho_attn_w"][:], state[r, "agg_in"][:])
            agg2 = sp.tile([128, 1], bf16, tag="agg2", bufs=2, name="agg2")
            nc.scalar.activation(agg2[:], ps_a[:, 0:1], Act.Relu, bias=wt["rho_attn_b"][:])
            ps_dot = pp.tile([4, 132], f32, tag="hh", bufs=1, name="ps_dot")
            nc.tensor.matmul(ps_dot[0:1, 0:H], agg2[:], wt["Vagg"][:])
            vxe = sp.tile([NF, H], bf16, tag="vxe", bufs=2, name="vxe")
            nc.vector.tensor_copy(vxe[:], wt["vxe_const"][:])
            nc.vector.tensor_copy(vxe[32:33, :], ps_dot[0:1, 0:H])
            state[r, "vxe"] = vxe

        def phase_tail(r):
            C, Tp, Text, NG = row_geom(r)
            xT, vxe, enc_tok = state[r, "xT"], state[r, "vxe"], state[r, "enc_tok"]
            ps_pre = pp.tile([128, 132], f32, tag="pre", bufs=1, name="ps_pre")
            for c in range(C + 1):
                nc.tensor.matmul(ps_pre[:, c * 4:(c + 1) * 4],
                                 xT[:, c * CH:(c + 1) * CH], vxe[:])
            e_tok = sp.tile([128, 132], bf16, tag="etok", bufs=2, name="e_tok")
            nc.scalar.activation(e_tok[:, 0:(C + 1) * 4], ps_pre[:, 0:(C + 1) * 4], Act.Exp)

            ps_hh = pp.tile([4, 132], f32, tag="hh", bufs=1, name="ps_hh")
            for c in range(C + 1):
                nc.tensor.matmul(ps_hh[:, 0:129], e_tok[:, c * 4:(c + 1) * 4],
                                 enc_tok[:, c, 0:129], start=(c == 0), stop=(c == C))
            rz = sp.tile([4, 1], f32, tag="rz", bufs=2, name="rz")
            nc.vector.reciprocal(rz[:], ps_hh[:, 128:129])
            hh_sb = sp.tile([4, 128], bf16, tag="hhsb", bufs=2, name="hh_sb")
            nc.vector.tensor_scalar(hh_sb[:], ps_hh[:, 0:128], rz[:], None, Alu.mult)
            ps_tr = pp.tile([128, 4], bf16, tag="pre", bufs=1, name="ps_tr")
            nc.tensor.transpose(ps_tr[:], hh_sb[:], ident[0:H, 0:H])
            nc.vector.tensor_copy(feat_all[:, r, :], ps_tr[:])
            for k in ("xT", "h1p1", "h2", "enc_tok", "vxe", "agg_in"):
                state.pop((r, k), None)

        # ---- software-pipelined emission: row r's tail hides under r+1's MLP
        phase_dma(0)
        phase_dma(1)
        for r in range(NSLOTS):
            phase_l1(r)
            phase_psi23(r)
            phase_phi2(r)
            phase_enc3(r)
            if r + 2 < NSLOTS:
                phase_dma(r + 2)
            if r >= 1:
                phase_tail(r - 1)
            phase_agg_pe(r)
        phase_tail(NSLOTS - 1)

        # ---- rho MLP over all 8 rows ----
        ps_r1 = pp.tile([128, 512], f32, tag="L2", bufs=2, name="ps_r1")
        for h in range(H):
            nc.tensor.matmul(ps_r1[:, 0:NSLOTS], wt["rho_w1"][:, h, :], feat_all[:, :, h],
                             start=(h == 0), stop=(h == H - 1))
        r1 = sp.tile([128, NSLOTS], bf16, tag="r1", name="r1")
        nc.scalar.activation(r1[:], ps_r1[:, 0:NSLOTS], Act.Relu, bias=wt["rho_b1"][:])
        ps_r2 = pp.tile([128, 512], f32, tag="L2", bufs=2, name="ps_r2")
        nc.tensor.matmul(ps_r2[:, 0:NSLOTS], wt["rho_w2"][:], r1[:])
        r2 = sp.tile([128, NSLOTS], bf16, tag="r2", name="r2")
        nc.scalar.activation(r2[:], ps_r2[:, 0:NSLOTS], Act.Relu, bias=wt["rho_b2"][:])
        ps_r3 = pp.tile([4, 132], f32, tag="hh", bufs=1, name="ps_r3")
        nc.tensor.matmul(ps_r3[0:1, 0:NSLOTS], wt["rho_w3"][:], r2[:])
        res = sp.tile([1, NSLOTS], f32, tag="res", name="res")
        nc.scalar.activation(res[:], ps_r3[0:1, 0:NSLOTS], Act.Sigmoid, bias=wt["rho_b3"][:])
        nc.sync.dma_start(out=d_out[:].rearrange("r one -> one r"), in_=res[:])
    return nc


def _patch_tile_drain(tile_mod, mybir):
    """Walrus in this env rejects >1 sync wait per instruction. Two fixes:
    1) split the Tile tail drain's waits across sequential drains;
    2) a post-pass over the final BIR that moves extra waits of ANY
       instruction onto standalone NoOps inserted just before it."""
    from concourse.vector_clock import ScopedClock
    if getattr(tile_mod.TileContext, "_drain_patched", False):
        return

    def _drain_and_barrier(self, tick_clock, wait_clock):
        nc = self.nc
        drain_inst = nc.sync.drain()
        wait_clock.add_sem_waits(drain_inst.ins, ScopedClock({None: tick_clock.global_clock}))
        si = drain_inst.ins.sync_info
        waits = list(si.on_wait or [])
        if len(waits) > 1:
            si.on_wait = waits[:1]
            for i in range(1, len(waits)):
                extra = nc.sync.drain()
                esi = extra.ins.sync_info
                if esi is None:
                    extra.ins.sync_info = mybir.SyncInfo(on_wait=waits[i:i + 1], on_update=[])
                else:
                    esi.on_wait = waits[i:i + 1]
        nc.all_engine_barrier()
        popped = nc._tile_sem_poison_stack.pop()
        assert popped is self._sem_poison
        nc.clear_and_free_semaphores(list(self.sems.allocated().values()))
        nc.all_engine_barrier()

    tile_mod.TileContext._drain_and_barrier = _drain_and_barrier

    _orig_exit = tile_mod.TileContext.__exit__

    def _exit(self, exc_type, exc_val, exc_tb):
        r = _orig_exit(self, exc_type, exc_val, exc_tb)
        if exc_type is None and getattr(tile_mod.TileContext, "_split_waits", True):
            _split_multi_waits(self.nc, mybir)
        return r

    def _split_multi_waits(nc, mybir):
        n = [0]
        for f in nc.m.functions:
            for bb in f.blocks:
                insts = bb.instructions
                out = []
                for inst in insts:
                    si = inst.sync_info
                    waits = list(si.on_wait) if (si and si.on_wait) else []
                    if len(waits) > 1:
                        for w in waits[:-1]:
                            n[0] += 1
                            nop = mybir.InstNoOp(name=f"I-ws-{n[0]}", ins=[], outs=[])
                            nop.engine = inst.engine
                            nop.sync_info = mybir.SyncInfo(on_wait=[w], on_update=[])
                            out.append(nop)
                        si.on_wait = waits[-1:]
                    out.append(inst)
                if len(out) != len(insts):
                    bb.instructions = out

    tile_mod.TileContext.__exit__ = _exit
    tile_mod.TileContext._drain_patched = True


_CACHE = {}
last_results = None


def _maybe_install_ntff_shim():
    """The image's antenv lacks axon_hooks; register the ctypes NTFF hook so
    run_bass_kernel_spmd(trace=True) can profile."""
    import sys, types
    if "antenv.axon_hooks" in sys.modules:
        return
    try:
        from trn_agent_boot.trn_boot import _ntff_profile_via_ctypes
        hook = _ntff_profile_via_ctypes("/opt/axon/libaxon_pjrt.so")
    except Exception:
        hook = None
    mod = types.ModuleType("antenv.axon_hooks")
    mod.get_axon_ntff_profile_hook = lambda: hook
    sys.modules["antenv.axon_hooks"] = mod


def _to_bf16(a):
    import ml_dtypes
    return np.asarray(a, np.float32).astype(ml_dtypes.bfloat16)


def _host_prep(inp):
    """Featurize + fold weights on host. Returns (const_map, xfeat, order, Cs)."""
    times = np.asarray(inp["times"], np.float32)[..., 0]      # [B,T]
    values = np.asarray(inp["values"], np.float32)[..., 0]    # [B,T]
    meas = np.asarray(inp["measurements"], np.int64)          # [B,T]
    lengths = np.asarray(inp["lengths"], np.int64)            # [B]

    ts = (MAXTS ** np.linspace(0.0, 1.0, NPOS // 2)).astype(np.float32)
    scaled = times[:, :, None] / ts[None, None, :]            # [B,T,8]
    xfeat = np.empty((B, NF, T), np.float32)
    xfeat[:, 0:8] = np.sin(scaled).transpose(0, 2, 1)
    xfeat[:, 8:16] = np.cos(scaled).transpose(0, 2, 1)
    xfeat[:, 16] = values
    oh = (meas[:, None, :] == np.arange(NMOD)[None, :, None])
    xfeat[:, 17:32] = oh.astype(np.float32)
    xfeat[:, 32] = 1.0
    xfeat[:, 33] = (np.arange(T)[None, :] >= lengths[:, None]).astype(np.float32)
    xfeat = _to_bf16(xfeat)

    f = lambda k: np.asarray(inp[k], np.float32)
    w1p = np.concatenate([f("phi_w1"), f("phi_b1")[None, :],
                          np.zeros((1, 128), np.float32)], 0)
    w1s = np.concatenate([f("psi_w1"), f("psi_b1")[None, :],
                          np.full((1, 128), -1e9, np.float32)], 0)
    wq = f("W_q") / np.sqrt(DP)
    Wk = f("W_k")
    Vx = np.stack([Wk[0:32, h * DP:(h + 1) * DP] @ wq[h] for h in range(H)], -1)
    Vagg = np.stack([Wk[32:, h * DP:(h + 1) * DP] @ wq[h] for h in range(H)], -1)
    vxe_const = np.zeros((NF, H), np.float32)
    vxe_const[0:32] = Vx
    vxe_const[33] = -1e9

    const_map = {
        "ident": _to_bf16(np.eye(128, dtype=np.float32)),
        "w1p": _to_bf16(w1p), "w1s": _to_bf16(w1s),
        "phi_w2": _to_bf16(f("phi_w2")), "psi_w2": _to_bf16(f("psi_w2")),
        "phi_w3": _to_bf16(f("phi_w3")), "psi_w3": _to_bf16(f("psi_w3")),
        "rho_attn_w": _to_bf16(f("rho_attn_w")), "Vagg": _to_bf16(Vagg),
        "vxe_const": _to_bf16(vxe_const),
        "demo_w1": _to_bf16(f("demo_w1")), "demo_w2": _to_bf16(f("demo_w2")),
        "rho_w1": np.ascontiguousarray(
            _to_bf16(f("rho_w1")).reshape(H, 128, 128).transpose(1, 0, 2)),
        "rho_w2": _to_bf16(f("rho_w2")), "rho_w3": _to_bf16(f("rho_w3")),
        "phi_b2": f("phi_b2"), "psi_b2": f("psi_b2"),
        "phi_b3": f("phi_b3"), "psi_b3": f("psi_b3"),
        "rho_attn_b": f("rho_attn_b"), "demo_b1": f("demo_b1"),
        "demo_b2": f("demo_b2"), "rho_b1": f("rho_b1"),
        "rho_b2": f("rho_b2"), "rho_b3": f("rho_b3"),
    }

    order = np.argsort(-lengths, kind="stable")
    Cs = []
    for s in range(NSLOTS):
        ranks = order[s * NCORES:(s + 1) * NCORES]
        Cs.append(int(np.ceil(lengths[ranks].max() / CH)))
    return const_map, xfeat, order, Cs, lengths


def kernel(**inputs):
    import os
    import concourse.bass as bass
    import concourse.mybir as mybir
    import concourse.tile as tile_mod
    from concourse import bass_utils

    _patch_tile_drain(tile_mod, mybir)

    inp = {k: np.asarray(v) for k, v in inputs.items()}
    const_map, xfeat, order, Cs, lengths = _host_prep(inp)
    demo = _to_bf16(np.asarray(inp["demo"], np.float32))

    key = (tuple(Cs), lengths.tobytes())
    ck = tuple(Cs)
    if ck not in _CACHE:
        _CACHE[ck] = _build_nc(Cs, tile_mod, bass, mybir)
    nc = _CACHE[ck]

    in_maps = []
    for core in range(NCORES):
        rows = [order[s * NCORES + core] for s in range(NSLOTS)]
        lens = lengths[rows].astype(np.int64)
        ninv = np.array([(Cs[s] + 1) * CH - int(lens[s]) - 1
                         for s in range(NSLOTS)], np.float32)
        m = {
            "xfeat_r": np.ascontiguousarray(xfeat[rows]),
            "demo_r": np.ascontiguousarray(demo[rows]),
            "ninv_neg": np.ascontiguousarray(
                np.broadcast_to(-ninv[None, :], (128, NSLOTS))),
            "recipL1": np.ascontiguousarray(np.broadcast_to(
                (1.0 / (lens + 1).astype(np.float32))[None, :], (128, NSLOTS))),
        }
        m.update(const_map)
        in_maps.append(m)

    trace = os.environ.get("KERNEL_TRACE", "0") == "1"
    kw = {}
    if trace:
        _maybe_install_ntff_shim()
        kw = dict(trace=True, tmpdir=os.environ.get("KERNEL_TRACE_DIR") or None)
    res = bass_utils.run_bass_kernel_spmd(nc, in_maps, core_ids=list(range(NCORES)), **kw)
    global last_results
    last_results = res
    out = np.zeros((B, 1), np.float32)
    for core in range(NCORES):
        for s in range(NSLOTS):
            out[order[s * NCORES + core], 0] = res.results[core]["out"][s, 0]
    return out


# revision 7
# speedup vs baseline: 1.0459x; 1.0459x over previous
"""Trainium2 Bass kernel for nn_DeepSetAttentionModel (segment_reduce) — v2.

Strategy (pure data parallel, 8 NeuronCores, host featurization):
- Host sorts the 64 set rows by length (desc); rank k -> core k%8, slot
  k//8. All cores run ONE SPMD program whose per-slot chunk counts are
  ceil(slot-max-length/128), so the program is identical across cores.
- Host precomputes the full 34-feature token matrix in feature-major
  (bf16): rows 0:16 sin/cos positional encodings, 16 value, 17:32
  one-hot, 32 const-1 (bias carrier), 33 invalid-flag. It is DMA'd
  straight into the feature-major xT layout the MLP matmuls consume:
  no on-device featurization, no x transposes, no Act-engine Sin.
- Layer-1 biases are folded into the weights via the const-1 row; the
  phi1/psi1 PSUM banks are adjacent so ONE Act relu covers both.
- psi of invalid tokens is forced to a known constant c3: psi_w1 gets
  an extra -1e9 row against the invalid flag (p1=0), and the resulting
  constant bias chain c3 = relu(psi_w3^T relu(psi_b2) + psi_b3) is
  removed with a host-known per-row count correction.
- preattn is computed token-major per 128-chunk ([34,128] stationary
  slices of xT against a [34,4] folded key vector whose row 32 carries
  the agg contribution and row 33 carries -1e9 for invalid tokens).
  |preattn| < 1 for this model, so exp needs no max-subtraction;
  invalid tokens underflow to exactly 0 and drop out of softmax sums.
- enc (phi output) is computed feature-major and moved token-major via
  xbar DMA transposes; the per-chunk weighted head sum contracts a
  129th all-ones column to produce the softmax normalizer for free.
"""
import numpy as np

B, T = 64, 4096
CH = 128
NPOS, V, NMOD = 16, 1, 15
NF = 34                      # 16 tt + 1 value + 15 onehot + 1 const + 1 inv
DP, H = 64, 4
MAXTS = 100.0
NCORES = 8
NSLOTS = B // NCORES


def _build_nc(Cs, tile_mod, bass, mybir):
    """Build the SPMD program for per-slot chunk counts Cs (len 8)."""
    f32 = mybir.dt.float32
    bf16 = mybir.dt.bfloat16
    Alu = mybir.AluOpType
    Act = mybir.ActivationFunctionType
    Cmax = max(Cs)
    NGmax = (((Cmax + 1) * CH) + 511) // 512

    nc = bass.Bass()
    dt_in = {}

    def din(name, shape, dtype=f32):
        dt_in[name] = nc.dram_tensor(name, list(shape), dtype, kind="ExternalInput")
        return dt_in[name]

    # per-core inputs
    d_xfeat = din("xfeat_r", [NSLOTS, NF, T], bf16)
    d_demo = din("demo_r", [NSLOTS, 8], bf16)
    din("ninv_neg", [128, NSLOTS])
    din("recipL1", [128, NSLOTS])
    # replicated constants / weights
    din("ident", [128, 128], bf16)
    WB = [("w1p", [NF, 128], bf16), ("w1s", [NF, 128], bf16),
          ("phi_w2", [128, 128], bf16), ("psi_w2", [128, 128], bf16),
          ("phi_w3", [128, 128], bf16), ("psi_w3", [128, 128], bf16),
          ("rho_attn_w", [128, 128], bf16), ("Vagg", [128, H], bf16),
          ("vxe_const", [NF, H], bf16),
          ("demo_w1", [8, 128], bf16), ("demo_w2", [128, 32], bf16),
          ("rho_w1", [128, H, 128], bf16), ("rho_w2", [128, 128], bf16),
          ("rho_w3", [128, 1], bf16)]
    for nm, shp, dt_ in WB:
        din(nm, shp, dt_)
    BIASES = [("phi_b2", 128), ("psi_b2", 128), ("phi_b3", 128), ("psi_b3", 128),
              ("rho_attn_b", 128), ("demo_b1", 128), ("demo_b2", 32),
              ("rho_b1", 128), ("rho_b2", 128), ("rho_b3", 1)]
    for nm, n in BIASES:
        din(nm, [n])
    d_out = nc.dram_tensor("out", [NSLOTS, 1], f32, kind="ExternalOutput")

    from contextlib import ExitStack
    with tile_mod.TileContext(nc) as tc, ExitStack() as stack:
        cp = stack.enter_context(tc.tile_pool(name="const", bufs=1))
        sp = stack.enter_context(tc.tile_pool(name="sbuf", bufs=1))
        pp = stack.enter_context(tc.tile_pool(name="psum", bufs=1, space="PSUM"))

        # ---- load constants / weights ----
        def load(name, shape, dtype=f32, ap=None):
            t = cp.tile(shape, dtype, tag=name, name=name)
            nc.sync.dma_start(out=t[:], in_=ap if ap is not None else dt_in[name][:])
            return t

        wt = {}
        for nm, shp, dt_ in WB:
            wt[nm] = load(nm, shp, dt_)
        for nm, n in BIASES:
            wt[nm] = load(nm, [n, 1], ap=dt_in[nm][:].unsqueeze(1))
        ident = load("ident", [128, 128], bf16)
        ninv_neg = load("ninv_neg", [128, NSLOTS])
        recipL1 = load("recipL1", [128, NSLOTS])
        demoT = load("demoT", [8, NSLOTS], bf16, ap=d_demo[:].rearrange("r f -> f r"))

        # ---- setup: demo encoder for all 8 slots -> demo_encT [34, 8] ----
        ps_d = pp.tile([128, 512], f32, tag="L2", bufs=2, name="ps_d")
        nc.tensor.matmul(ps_d[:, 0:NSLOTS], wt["demo_w1"][:], demoT[:])
        dh1 = sp.tile([128, NSLOTS], bf16, tag="dh1", name="dh1")
        nc.scalar.activation(dh1[:], ps_d[:, 0:NSLOTS], Act.Relu, bias=wt["demo_b1"][:])
        ps_d2 = pp.tile([128, 512], f32, tag="L2", bufs=2, name="ps_d2")
        nc.tensor.matmul(ps_d2[0:32, 0:NSLOTS], wt["demo_w2"][:], dh1[:])
        demo_encT = cp.tile([NF, NSLOTS], bf16, tag="demo_encT", name="demo_encT")
        nc.scalar.activation(demo_encT[0:32, :], ps_d2[0:32, 0:NSLOTS],
                             Act.Identity, bias=wt["demo_b2"][:])
        nc.gpsimd.memset(demo_encT[32:34, :], 0.0)
        nc.gpsimd.memset(demo_encT[32:33, :], 1.0)

        # ---- setup: c3 = relu(psi_w3^T relu(psi_b2) + psi_b3), correction ----
        p2c = sp.tile([128, 1], bf16, tag="p2c", name="p2c")
        nc.scalar.activation(p2c[:], wt["psi_b2"][:], Act.Relu)
        ps_c3 = pp.tile([128, 512], f32, tag="L2", bufs=2, name="ps_c3")
        nc.tensor.matmul(ps_c3[:, 0:1], wt["psi_w3"][:], p2c[:])
        c3 = sp.tile([128, 1], f32, tag="c3", name="c3")
        nc.scalar.activation(c3[:], ps_c3[:, 0:1], Act.Relu, bias=wt["psi_b3"][:])
        negc3nr = cp.tile([128, NSLOTS], f32, tag="negc3nr", name="negc3nr")
        nc.vector.tensor_tensor(out=negc3nr[:], in0=c3[:].to_broadcast([128, NSLOTS]),
                                in1=ninv_neg[:], op=Alu.mult)

        feat_all = sp.tile([128, NSLOTS, H], bf16, tag="feat_all", name="feat_all")

        # ---- per-row phases (emitted software-pipelined below) ----
        state = {}

        def row_geom(r):
            C = Cs[r]
            return C, C * CH, (C + 1) * CH, ((C + 1) * CH + 511) // 512

        def phase_dma(r):
            C, Tp, Text, NG = row_geom(r)
            xT = sp.tile([NF, (Cmax + 1) * CH], bf16, tag="xT", bufs=3, name="xT")
            nc.sync.dma_start(out=xT[:, 0:Tp], in_=d_xfeat[r, :, 0:Tp])
            nc.gpsimd.memset(xT[:, Tp + 1:Text], 0.0)
            # rows 32/33 (const, inv) both 1.0 in the padding: the -1e9 inv
            # weight dominates, so a stray aggdot via the const row is inert
            nc.gpsimd.memset(xT[32:34, Tp + 1:Text], 1.0)
            nc.vector.tensor_copy(xT[:, Tp:Tp + 1], demo_encT[:, r:r + 1])
            state[r, "xT"] = xT

        def phase_l1(r):
            C, Tp, Text, NG = row_geom(r)
            xT = state[r, "xT"]
            h1p1 = sp.tile([128, NGmax * 1024], bf16, tag="h1p1", bufs=2, name="h1p1")
            for g in range(NG):
                N = min(512, Text - g * 512)
                ps = pp.tile([128, 1024], f32, tag="L1", bufs=2, name="ps_l1")
                nc.tensor.matmul(ps[:, 0:N], wt["w1p"][:], xT[:, g * 512:g * 512 + N])
                nc.tensor.matmul(ps[:, 512:512 + N], wt["w1s"][:], xT[:, g * 512:g * 512 + N])
                if N == 512:
                    nc.scalar.activation(h1p1[:, g * 1024:(g + 1) * 1024], ps[:], Act.Relu)
                else:
                    nc.scalar.activation(h1p1[:, g * 1024:g * 1024 + N], ps[:, 0:N], Act.Relu)
                    nc.scalar.activation(h1p1[:, g * 1024 + 512:g * 1024 + 512 + N],
                                         ps[:, 512:512 + N], Act.Relu)
            state[r, "h1p1"] = h1p1

        def phase_psi23(r):
            C, Tp, Text, NG = row_geom(r)
            h1p1 = state[r, "h1p1"]
            p2 = sp.tile([128, (Cmax + 1) * CH], bf16, tag="p2", bufs=2, name="p2")
            for g in range(NG):
                N = min(512, Text - g * 512)
                ps = pp.tile([128, 512], f32, tag="L2", bufs=2, name="ps_psi2")
                nc.tensor.matmul(ps[:, 0:N], wt["psi_w2"][:],
                                 h1p1[:, g * 1024 + 512:g * 1024 + 512 + N])
                nc.vector.tensor_scalar(p2[:, g * 512:g * 512 + N], ps[:, 0:N],
                                        wt["psi_b2"][:], 0.0, Alu.add, Alu.max)
            acc = sp.tile([128, 16], f32, tag="acc", bufs=2, name="acc")
            psi3s = sp.tile([128, 512], bf16, tag="psi3s", bufs=2, name="psi3s")
            for g in range(NG):
                N = min(512, Text - g * 512)
                ps = pp.tile([128, 512], f32, tag="L2", bufs=2, name="ps_psi3")
                nc.tensor.matmul(ps[:, 0:N], wt["psi_w3"][:], p2[:, g * 512:g * 512 + N])
                nc.scalar.activation(psi3s[:, 0:N], ps[:, 0:N], Act.Relu,
                                     bias=wt["psi_b3"][:], accum_out=acc[:, g:g + 1])
            # agg DVE part (runs while PE continues with phi2/enc3)
            aggs = sp.tile([128, 1], f32, tag="aggs", bufs=2, name="aggs")
            nc.vector.tensor_reduce(aggs[:], acc[:, 0:NG], mybir.AxisListType.X, Alu.add)
            agg_in = sp.tile([128, 1], bf16, tag="aggin", bufs=2, name="agg_in")
            nc.vector.tensor_scalar(agg_in[:], aggs[:], negc3nr[:, r:r + 1],
                                    recipL1[:, r:r + 1], Alu.add, Alu.mult)
            state[r, "agg_in"] = agg_in

        def phase_phi2(r):
            C, Tp, Text, NG = row_geom(r)
            h1p1 = state[r, "h1p1"]
            h2 = sp.tile([128, (Cmax + 1) * CH], bf16, tag="h2", bufs=2, name="h2")
            for g in range(NG):
                N = min(512, Text - g * 512)
                ps = pp.tile([128, 512], f32, tag="L2", bufs=2, name="ps_phi2")
                nc.tensor.matmul(ps[:, 0:N], wt["phi_w2"][:], h1p1[:, g * 1024:g * 1024 + N])
                nc.vector.tensor_scalar(h2[:, g * 512:g * 512 + N], ps[:, 0:N],
                                        wt["phi_b2"][:], 0.0, Alu.add, Alu.max)
            state[r, "h2"] = h2

        def phase_enc3(r):
            C, Tp, Text, NG = row_geom(r)
            h2 = state[r, "h2"]
            enc_tok = sp.tile([128, Cmax + 1, 144], bf16, tag="enctok", bufs=2, name="enc_tok")
            nc.gpsimd.memset(enc_tok[:, 0:C + 1, 128:129], 1.0)
            for g in range(NG):
                N = min(512, Text - g * 512)
                ps = pp.tile([128, 512], f32, tag="L2", bufs=2, name="ps_phi3")
                nc.tensor.matmul(ps[:, 0:N], wt["phi_w3"][:], h2[:, g * 512:g * 512 + N])
                encfm = sp.tile([128, 512], bf16, tag="encfm", bufs=2, name="encfm")
                nc.vector.tensor_scalar(encfm[:, 0:N], ps[:, 0:N],
                                        wt["phi_b3"][:], 0.0, Alu.add, Alu.max)
                c0 = g * 4
                nch = min(4, C + 1 - c0)
                nc.sync.dma_start_transpose(out=enc_tok[:, c0:c0 + nch, 0:128],
                                            in_=encfm[:, 0:nch * CH])
            state[r, "enc_tok"] = enc_tok

        def phase_agg_pe(r):
            ps_a = pp.tile([128, 512], f32, tag="L2", bufs=2, name="ps_a")
            nc.tensor.matmul(ps_a[:, 0:1], wt["rho_attn_w"][:], state[r, "agg_in"][:])
            agg2 = sp.tile([128, 1], bf16, tag="agg2", bufs=2, name="agg2")
            nc.scalar.activation(agg2[:], ps_a[:, 0:1], Act.Relu, bias=wt["rho_attn_b"][:])
            ps_dot = pp.tile([4, 132], f32, tag="hh", bufs=1, name="ps_dot")
            nc.tensor.matmul(ps_dot[0:1, 0:H], agg2[:], wt["Vagg"][:])
            vxe = sp.tile([NF, H], bf16, tag="vxe", bufs=2, name="vxe")
            nc.vector.tensor_copy(vxe[:], wt["vxe_const"][:])
            nc.vector.tensor_copy(vxe[32:33, :], ps_dot[0:1, 0:H])
            state[r, "vxe"] = vxe

        def phase_tail(r):
            C, Tp, Text, NG = row_geom(r)
            xT, vxe, enc_tok = state[r, "xT"], state[r, "vxe"], state[r, "enc_tok"]
            ps_pre = pp.tile([128, 132], f32, tag="pre", bufs=1, name="ps_pre")
            for c in range(C + 1):
                nc.tensor.matmul(ps_pre[:, c * 4:(c + 1) * 4],
                                 xT[:, c * CH:(c + 1) * CH], vxe[:])
            e_tok = sp.tile([128, 132], bf16, tag="etok", bufs=2, name="e_tok")
            nc.scalar.activation(e_tok[:, 0:(C + 1) * 4], ps_pre[:, 0:(C + 1) * 4], Act.Exp)

            ps_hh = pp.tile([4, 132], f32, tag="hh", bufs=1, name="ps_hh")
            for c in range(C + 1):
                nc.tensor.matmul(ps_hh[:, 0:129], e_tok[:, c * 4:(c + 1) * 4],
                                 enc_tok[:, c, 0:129], start=(c == 0), stop=(c == C))
            rz = sp.tile([4, 1], f32, tag="rz", bufs=2, name="rz")
            nc.vector.reciprocal(rz[:], ps_hh[:, 128:129])
            hh_sb = sp.tile([4, 128], bf16, tag="hhsb", bufs=2, name="hh_sb")
            nc.vector.tensor_scalar(hh_sb[:], ps_hh[:, 0:128], rz[:], None, Alu.mult)
            ps_tr = pp.tile([128, 4], bf16, tag="pre", bufs=1, name="ps_tr")
            nc.tensor.transpose(ps_tr[:], hh_sb[:], ident[0:H, 0:H])
            nc.vector.tensor_copy(feat_all[:, r, :], ps_tr[:])
            for k in ("xT", "h1p1", "h2", "enc_tok", "vxe", "agg_in"):
                state.pop((r, k), None)

        # ---- software-pipelined emission: row r's tail hides under r+1's MLP
        phase_dma(0)
        phase_dma(1)
        for r in range(NSLOTS):
            phase_l1(r)
            phase_psi23(r)
            phase_phi2(r)
            phase_enc3(r)
            if r >= 1:
                phase_tail(r - 1)
            if r + 2 < NSLOTS:
                phase_dma(r + 2)
            phase_agg_pe(r)
        phase_tail(NSLOTS - 1)

        # ---- rho MLP over all 8 rows ----
        ps_r1 = pp.tile([128, 512], f32, tag="L2", bufs=2, name="ps_r1")
        for h in range(H):
            nc.tensor.matmul(ps_r1[:, 0:NSLOTS], wt["rho_w1"][:, h, :], feat_all[:, :, h],
                             start=(h == 0), stop=(h == H - 1))
        r1 = sp.tile([128, NSLOTS], bf16, tag="r1", name="r1")
        nc.scalar.activation(r1[:], ps_r1[:, 0:NSLOTS], Act.Relu, bias=wt["rho_b1"][:])
        ps_r2 = pp.tile([128, 512], f32, tag="L2", bufs=2, name="ps_r2")
        nc.tensor.matmul(ps_r2[:, 0:NSLOTS], wt["rho_w2"][:], r1[:])
        r2 = sp.tile([128, NSLOTS], bf16, tag="r2", name="r2")
        nc.scalar.activation(r2[:], ps_r2[:, 0:NSLOTS], Act.Relu, bias=wt["rho_b2"][:])
        ps_r3 = pp.tile([4, 132], f32, tag="hh", bufs=1, name="ps_r3")
        nc.tensor.matmul(ps_r3[0:1, 0:NSLOTS], wt["rho_w3"][:], r2[:])
        res = sp.tile([1, NSLOTS], f32, tag="res", name="res")
        nc.scalar.activation(res[:], ps_r3[0:1, 0:NSLOTS], Act.Sigmoid, bias=wt["rho_b3"][:])
        nc.sync.dma_start(out=d_out[:].rearrange("r one -> one r"), in_=res[:])
    return nc


def _patch_tile_drain(tile_mod, mybir):
    """Walrus in this env rejects >1 sync wait per instruction. Two fixes:
    1) split the Tile tail drain's waits across sequential drains;
    2) a post-pass over the final BIR that moves extra waits of ANY
       instruction onto standalone NoOps inserted just before it."""
    from concourse.vector_clock import ScopedClock
    if getattr(tile_mod.TileContext, "_drain_patched", False):
        return

    def _drain_and_barrier(self, tick_clock, wait_clock):
        nc = self.nc
        drain_inst = nc.sync.drain()
        wait_clock.add_sem_waits(drain_inst.ins, ScopedClock({None: tick_clock.global_clock}))
        si = drain_inst.ins.sync_info
        waits = list(si.on_wait or [])
        if len(waits) > 1:
            si.on_wait = waits[:1]
            for i in range(1, len(waits)):
                extra = nc.sync.drain()
                esi = extra.ins.sync_info
                if esi is None:
                    extra.ins.sync_info = mybir.SyncInfo(on_wait=waits[i:i + 1], on_update=[])
                else:
                    esi.on_wait = waits[i:i + 1]
        nc.all_engine_barrier()
        popped = nc._tile_sem_poison_stack.pop()
        assert popped is self._sem_poison
        nc.clear_and_free_semaphores(list(self.sems.allocated().values()))
        nc.all_engine_barrier()

    tile_mod.TileContext._drain_and_barrier = _drain_and_barrier

    _orig_exit = tile_mod.TileContext.__exit__

    def _exit(self, exc_type, exc_val, exc_tb):
        r = _orig_exit(self, exc_type, exc_val, exc_tb)
        if exc_type is None and getattr(tile_mod.TileContext, "_split_waits", True):
            _split_multi_waits(self.nc, mybir)
        return r

    def _split_multi_waits(nc, mybir):
        n = [0]
        for f in nc.m.functions:
            for bb in f.blocks:
                insts = bb.instructions
                out = []
                for inst in insts:
                    si = inst.sync_info
                    waits = list(si.on_wait) if (si and si.on_wait) else []
                    if len(waits) > 1:
                        for w in waits[:-1]:
                            n[0] += 1
                            nop = mybir.InstNoOp(name=f"I-ws-{n[0]}", ins=[], outs=[])
                            nop.engine = inst.engine
                            nop.sync_info = mybir.SyncInfo(on_wait=[w], on_update=[])
                            out.append(nop)
                        si.on_wait = waits[-1:]
                    out.append(inst)
                if len(out) != len(insts):
                    bb.instructions = out

    tile_mod.TileContext.__exit__ = _exit
    tile_mod.TileContext._drain_patched = True


_CACHE = {}
last_results = None


def _maybe_install_ntff_shim():
    """The image's antenv lacks axon_hooks; register the ctypes NTFF hook so
    run_bass_kernel_spmd(trace=True) can profile."""
    import sys, types
    if "antenv.axon_hooks" in sys.modules:
        return
    try:
        from trn_agent_boot.trn_boot import _ntff_profile_via_ctypes
        hook = _ntff_profile_via_ctypes("/opt/axon/libaxon_pjrt.so")
    except Exception:
        hook = None
    mod = types.ModuleType("antenv.axon_hooks")
    mod.get_axon_ntff_profile_hook = lambda: hook
    sys.modules["antenv.axon_hooks"] = mod


def _to_bf16(a):
    import ml_dtypes
    return np.asarray(a, np.float32).astype(ml_dtypes.bfloat16)


def _host_prep(inp):
    """Featurize + fold weights on host. Returns (const_map, xfeat, order, Cs)."""
    times = np.asarray(inp["times"], np.float32)[..., 0]      # [B,T]
    values = np.asarray(inp["values"], np.float32)[..., 0]    # [B,T]
    meas = np.asarray(inp["measurements"], np.int64)          # [B,T]
    lengths = np.asarray(inp["lengths"], np.int64)            # [B]

    ts = (MAXTS ** np.linspace(0.0, 1.0, NPOS // 2)).astype(np.float32)
    scaled = times[:, :, None] / ts[None, None, :]            # [B,T,8]
    xfeat = np.empty((B, NF, T), np.float32)
    xfeat[:, 0:8] = np.sin(scaled).transpose(0, 2, 1)
    xfeat[:, 8:16] = np.cos(scaled).transpose(0, 2, 1)
    xfeat[:, 16] = values
    oh = (meas[:, None, :] == np.arange(NMOD)[None, :, None])
    xfeat[:, 17:32] = oh.astype(np.float32)
    xfeat[:, 32] = 1.0
    xfeat[:, 33] = (np.arange(T)[None, :] >= lengths[:, None]).astype(np.float32)
    xfeat = _to_bf16(xfeat)

    f = lambda k: np.asarray(inp[k], np.float32)
    w1p = np.concatenate([f("phi_w1"), f("phi_b1")[None, :],
                          np.zeros((1, 128), np.float32)], 0)
    w1s = np.concatenate([f("psi_w1"), f("psi_b1")[None, :],
                          np.full((1, 128), -1e9, np.float32)], 0)
    wq = f("W_q") / np.sqrt(DP)
    Wk = f("W_k")
    Vx = np.stack([Wk[0:32, h * DP:(h + 1) * DP] @ wq[h] for h in range(H)], -1)
    Vagg = np.stack([Wk[32:, h * DP:(h + 1) * DP] @ wq[h] for h in range(H)], -1)
    vxe_const = np.zeros((NF, H), np.float32)
    vxe_const[0:32] = Vx
    vxe_const[33] = -1e9

    const_map = {
        "ident": _to_bf16(np.eye(128, dtype=np.float32)),
        "w1p": _to_bf16(w1p), "w1s": _to_bf16(w1s),
        "phi_w2": _to_bf16(f("phi_w2")), "psi_w2": _to_bf16(f("psi_w2")),
        "phi_w3": _to_bf16(f("phi_w3")), "psi_w3": _to_bf16(f("psi_w3")),
        "rho_attn_w": _to_bf16(f("rho_attn_w")), "Vagg": _to_bf16(Vagg),
        "vxe_const": _to_bf16(vxe_const),
        "demo_w1": _to_bf16(f("demo_w1")), "demo_w2": _to_bf16(f("demo_w2")),
        "rho_w1": np.ascontiguousarray(
            _to_bf16(f("rho_w1")).reshape(H, 128, 128).transpose(1, 0, 2)),
        "rho_w2": _to_bf16(f("rho_w2")), "rho_w3": _to_bf16(f("rho_w3")),
        "phi_b2": f("phi_b2"), "psi_b2": f("psi_b2"),
        "phi_b3": f("phi_b3"), "psi_b3": f("psi_b3"),
        "rho_attn_b": f("rho_attn_b"), "demo_b1": f("demo_b1"),
        "demo_b2": f("demo_b2"), "rho_b1": f("rho_b1"),
        "rho_b2": f("rho_b2"), "rho_b3": f("rho_b3"),
    }

    order = np.argsort(-lengths, kind="stable")
    Cs = []
    for s in range(NSLOTS):
        ranks = order[s * NCORES:(s + 1) * NCORES]
        Cs.append(int(np.ceil(lengths[ranks].max() / CH)))
    return const_map, xfeat, order, Cs, lengths


def kernel(**inputs):
    import os
    import concourse.bass as bass
    import concourse.mybir as mybir
    import concourse.tile as tile_mod
    from concourse import bass_utils

    _patch_tile_drain(tile_mod, mybir)

    inp = {k: np.asarray(v) for k, v in inputs.items()}
    const_map, xfeat, order, Cs, lengths = _host_prep(inp)
    demo = _to_bf16(np.asarray(inp["demo"], np.float32))

    key = (tuple(Cs), lengths.tobytes())
    ck = tuple(Cs)
    if ck not in _CACHE:
        _CACHE[ck] = _build_nc(Cs, tile_mod, bass, mybir)
    nc = _CACHE[ck]

    in_maps = []
    for core in range(NCORES):
        rows = [order[s * NCORES + core] for s in range(NSLOTS)]
        lens = lengths[rows].astype(np.int64)
        ninv = np.array([(Cs[s] + 1) * CH - int(lens[s]) - 1
                         for s in range(NSLOTS)], np.float32)
        m = {
            "xfeat_r": np.ascontiguousarray(xfeat[rows]),
            "demo_r": np.ascontiguousarray(demo[rows]),
            "ninv_neg": np.ascontiguousarray(
                np.broadcast_to(-ninv[None, :], (128, NSLOTS))),
            "recipL1": np.ascontiguousarray(np.broadcast_to(
                (1.0 / (lens + 1).astype(np.float32))[None, :], (128, NSLOTS))),
        }
        m.update(const_map)
        in_maps.append(m)

    trace = os.environ.get("KERNEL_TRACE", "0") == "1"
    kw = {}
    if trace:
        _maybe_install_ntff_shim()
        kw = dict(trace=True, tmpdir=os.environ.get("KERNEL_TRACE_DIR") or None)
    res = bass_utils.run_bass_kernel_spmd(nc, in_maps, core_ids=list(range(NCORES)), **kw)
    global last_results
    last_results = res
    out = np.zeros((B, 1), np.float32)
    for core in range(NCORES):
        for s in range(NSLOTS):
            out[order[s * NCORES + core], 0] = res.results[core]["out"][s, 0]
    return out


# revision 10
# speedup vs baseline: 1.0638x; 1.0171x over previous
"""Trainium2 Bass kernel for nn_DeepSetAttentionModel (segment_reduce) — v2.

Strategy (pure data parallel, 8 NeuronCores, host featurization):
- Host sorts the 64 set rows by length (desc); rank k -> core k%8, slot
  k//8. All cores run ONE SPMD program whose per-slot chunk counts are
  ceil(slot-max-length/128), so the program is identical across cores.
- Host precomputes the full 34-feature token matrix in feature-major
  (bf16): rows 0:16 sin/cos positional encodings, 16 value, 17:32
  one-hot, 32 const-1 (bias carrier), 33 invalid-flag. It is DMA'd
  straight into the feature-major xT layout the MLP matmuls consume:
  no on-device featurization, no x transposes, no Act-engine Sin.
- Layer-1 biases are folded into the weights via the const-1 row; the
  phi1/psi1 PSUM banks are adjacent so ONE Act relu covers both.
- psi of invalid tokens is forced to a known constant c3: psi_w1 gets
  an extra -1e9 row against the invalid flag (p1=0), and the resulting
  constant bias chain c3 = relu(psi_w3^T relu(psi_b2) + psi_b3) is
  removed with a host-known per-row count correction.
- preattn is computed token-major per 128-chunk ([34,128] stationary
  slices of xT against a [34,4] folded key vector whose row 32 carries
  the agg contribution and row 33 carries -1e9 for invalid tokens).
  |preattn| < 1 for this model, so exp needs no max-subtraction;
  invalid tokens underflow to exactly 0 and drop out of softmax sums.
- enc (phi output) is computed feature-major and moved token-major via
  xbar DMA transposes; the per-chunk weighted head sum contracts a
  129th all-ones column to produce the softmax normalizer for free.
"""
import numpy as np

B, T = 64, 4096
CH = 128
NPOS, V, NMOD = 16, 1, 15
NF = 34                      # 16 tt + 1 value + 15 onehot + 1 const + 1 inv
DP, H = 64, 4
MAXTS = 100.0
NCORES = 8
NSLOTS = B // NCORES


def _build_nc(Cs, tile_mod, bass, mybir):
    """Build the SPMD program for per-slot chunk counts Cs (len 8)."""
    f32 = mybir.dt.float32
    bf16 = mybir.dt.bfloat16
    Alu = mybir.AluOpType
    Act = mybir.ActivationFunctionType
    Cmax = max(Cs)
    NGmax = (((Cmax + 1) * CH) + 511) // 512

    nc = bass.Bass()
    dt_in = {}

    def din(name, shape, dtype=f32):
        dt_in[name] = nc.dram_tensor(name, list(shape), dtype, kind="ExternalInput")
        return dt_in[name]

    # per-core inputs
    d_xfeat = din("xfeat_r", [NSLOTS, NF, T], bf16)
    d_demo = din("demo_r", [NSLOTS, 8], bf16)
    din("ninv_neg", [128, NSLOTS])
    din("recipL1", [128, NSLOTS])
    # replicated constants / weights
    din("ident", [128, 128], bf16)
    WB = [("w1p", [NF, 128], bf16), ("w1s", [NF, 128], bf16),
          ("phi_w2", [128, 128], bf16), ("psi_w2", [128, 128], bf16),
          ("phi_w3", [128, 128], bf16), ("psi_w3", [128, 128], bf16),
          ("rho_attn_w", [128, 128], bf16), ("Vagg", [128, H], bf16),
          ("vxe_const", [NF, H], bf16),
          ("demo_w1", [8, 128], bf16), ("demo_w2", [128, 32], bf16),
          ("rho_w1", [128, H, 128], bf16), ("rho_w2", [128, 128], bf16),
          ("rho_w3", [128, 1], bf16)]
    for nm, shp, dt_ in WB:
        din(nm, shp, dt_)
    BIASES = [("phi_b2", 128), ("psi_b2", 128), ("phi_b3", 128), ("psi_b3", 128),
              ("rho_attn_b", 128), ("demo_b1", 128), ("demo_b2", 32),
              ("rho_b1", 128), ("rho_b2", 128), ("rho_b3", 1)]
    for nm, n in BIASES:
        din(nm, [n])
    d_out = nc.dram_tensor("out", [NSLOTS, 1], f32, kind="ExternalOutput")

    from contextlib import ExitStack
    with tile_mod.TileContext(nc) as tc, ExitStack() as stack:
        cp = stack.enter_context(tc.tile_pool(name="const", bufs=1))
        sp = stack.enter_context(tc.tile_pool(name="sbuf", bufs=1))
        pp = stack.enter_context(tc.tile_pool(name="psum", bufs=1, space="PSUM"))

        # ---- load constants / weights ----
        def load(name, shape, dtype=f32, ap=None):
            t = cp.tile(shape, dtype, tag=name, name=name)
            nc.sync.dma_start(out=t[:], in_=ap if ap is not None else dt_in[name][:])
            return t

        wt = {}
        for nm, shp, dt_ in WB:
            wt[nm] = load(nm, shp, dt_)
        for nm, n in BIASES:
            wt[nm] = load(nm, [n, 1], ap=dt_in[nm][:].unsqueeze(1))
        ident = load("ident", [128, 128], bf16)
        ninv_neg = load("ninv_neg", [128, NSLOTS])
        recipL1 = load("recipL1", [128, NSLOTS])
        demoT = load("demoT", [8, NSLOTS], bf16, ap=d_demo[:].rearrange("r f -> f r"))

        # ---- setup: demo encoder for all 8 slots -> demo_encT [34, 8] ----
        ps_d = pp.tile([128, 512], f32, tag="L2", bufs=3, name="ps_d")
        nc.tensor.matmul(ps_d[:, 0:NSLOTS], wt["demo_w1"][:], demoT[:])
        dh1 = sp.tile([128, NSLOTS], bf16, tag="dh1", name="dh1")
        nc.scalar.activation(dh1[:], ps_d[:, 0:NSLOTS], Act.Relu, bias=wt["demo_b1"][:])
        ps_d2 = pp.tile([128, 512], f32, tag="L2", bufs=3, name="ps_d2")
        nc.tensor.matmul(ps_d2[0:32, 0:NSLOTS], wt["demo_w2"][:], dh1[:])
        demo_encT = cp.tile([NF, NSLOTS], bf16, tag="demo_encT", name="demo_encT")
        nc.scalar.activation(demo_encT[0:32, :], ps_d2[0:32, 0:NSLOTS],
                             Act.Identity, bias=wt["demo_b2"][:])
        nc.gpsimd.memset(demo_encT[32:34, :], 0.0)
        nc.gpsimd.memset(demo_encT[32:33, :], 1.0)

        # ---- setup: c3 = relu(psi_w3^T relu(psi_b2) + psi_b3), correction ----
        p2c = sp.tile([128, 1], bf16, tag="p2c", name="p2c")
        nc.scalar.activation(p2c[:], wt["psi_b2"][:], Act.Relu)
        ps_c3 = pp.tile([128, 512], f32, tag="L2", bufs=3, name="ps_c3")
        nc.tensor.matmul(ps_c3[:, 0:1], wt["psi_w3"][:], p2c[:])
        c3 = sp.tile([128, 1], f32, tag="c3", name="c3")
        nc.scalar.activation(c3[:], ps_c3[:, 0:1], Act.Relu, bias=wt["psi_b3"][:])
        negc3nr = cp.tile([128, NSLOTS], f32, tag="negc3nr", name="negc3nr")
        nc.vector.tensor_tensor(out=negc3nr[:], in0=c3[:].to_broadcast([128, NSLOTS]),
                                in1=ninv_neg[:], op=Alu.mult)

        feat_all = sp.tile([128, NSLOTS, H], bf16, tag="feat_all", name="feat_all")

        # ---- per-row phases (emitted software-pipelined below) ----
        state = {}
        eng_load = [0.0, 0.0]  # running Act / DVE load estimate (us)

        def relu_store(out_ap, in_ap, bias_ap):
            """PSUM->SBUF relu on whichever of Act/DVE is less loaded."""
            if eng_load[0] + 0.72 <= eng_load[1] + 0.658:
                eng_load[0] += 0.72
                nc.scalar.activation(out_ap, in_ap, Act.Relu, bias=bias_ap)
            else:
                eng_load[1] += 0.658
                nc.vector.tensor_scalar(out_ap, in_ap, bias_ap, 0.0, Alu.add, Alu.max)

        def row_geom(r):
            C = Cs[r]
            return C, C * CH, (C + 1) * CH, ((C + 1) * CH + 511) // 512

        def phase_dma(r):
            C, Tp, Text, NG = row_geom(r)
            xT = sp.tile([NF, (Cmax + 1) * CH], bf16, tag="xT", bufs=3, name="xT")
            nc.sync.dma_start(out=xT[:, 0:Tp], in_=d_xfeat[r, :, 0:Tp])
            nc.gpsimd.memset(xT[:, Tp + 1:Text], 0.0)
            # rows 32/33 (const, inv) both 1.0 in the padding: the -1e9 inv
            # weight dominates, so a stray aggdot via the const row is inert
            nc.gpsimd.memset(xT[32:34, Tp + 1:Text], 1.0)
            nc.vector.tensor_copy(xT[:, Tp:Tp + 1], demo_encT[:, r:r + 1])
            state[r, "xT"] = xT

        def mlp_units(r):
            """Yield per-group emit closures for row r's MLP, interleaved so
            consecutive PE matmuls feed alternating relu consumers."""
            C, Tp, Text, NG = row_geom(r)
            xT = state[r, "xT"]
            h1p1 = sp.tile([128, NGmax * 1024], bf16, tag="h1p1", bufs=2, name="h1p1")
            h2 = sp.tile([128, (Cmax + 1) * CH], bf16, tag="h2", bufs=2, name="h2")
            p2 = sp.tile([128, (Cmax + 1) * CH], bf16, tag="p2", bufs=2, name="p2")
            enc_tok = sp.tile([128, Cmax + 1, 144], bf16, tag="enctok", bufs=2, name="enc_tok")
            acc = sp.tile([128, 16], f32, tag="acc", bufs=2, name="acc")
            psi3s = sp.tile([128, 512], bf16, tag="psi3s", bufs=2, name="psi3s")
            state[r, "enc_tok"] = enc_tok

            def u_l1(g):
                N = min(512, Text - g * 512)
                ps = pp.tile([128, 1024], f32, tag="L1", bufs=2, name="ps_l1")
                nc.tensor.matmul(ps[:, 0:N], wt["w1p"][:], xT[:, g * 512:g * 512 + N])
                nc.tensor.matmul(ps[:, 512:512 + N], wt["w1s"][:], xT[:, g * 512:g * 512 + N])
                eng_load[0] += 1.0
                if N == 512:
                    nc.scalar.activation(h1p1[:, g * 1024:(g + 1) * 1024], ps[:], Act.Relu)
                else:
                    nc.scalar.activation(h1p1[:, g * 1024:g * 1024 + N], ps[:, 0:N], Act.Relu)
                    nc.scalar.activation(h1p1[:, g * 1024 + 512:g * 1024 + 512 + N],
                                         ps[:, 512:512 + N], Act.Relu)

            def u_psi2(g):
                N = min(512, Text - g * 512)
                ps = pp.tile([128, 512], f32, tag="L2", bufs=3, name="ps_psi2")
                nc.tensor.matmul(ps[:, 0:N], wt["psi_w2"][:],
                                 h1p1[:, g * 1024 + 512:g * 1024 + 512 + N])
                relu_store(p2[:, g * 512:g * 512 + N], ps[:, 0:N], wt["psi_b2"][:])

            def u_phi2(g):
                N = min(512, Text - g * 512)
                ps = pp.tile([128, 512], f32, tag="L2", bufs=3, name="ps_phi2")
                nc.tensor.matmul(ps[:, 0:N], wt["phi_w2"][:], h1p1[:, g * 1024:g * 1024 + N])
                relu_store(h2[:, g * 512:g * 512 + N], ps[:, 0:N], wt["phi_b2"][:])

            def u_psi3(g):
                N = min(512, Text - g * 512)
                ps = pp.tile([128, 512], f32, tag="L2", bufs=3, name="ps_psi3")
                nc.tensor.matmul(ps[:, 0:N], wt["psi_w3"][:], p2[:, g * 512:g * 512 + N])
                eng_load[0] += 0.72
                nc.scalar.activation(psi3s[:, 0:N], ps[:, 0:N], Act.Relu,
                                     bias=wt["psi_b3"][:], accum_out=acc[:, g:g + 1])

            def u_phi3(g):
                N = min(512, Text - g * 512)
                ps = pp.tile([128, 512], f32, tag="L2", bufs=3, name="ps_phi3")
                nc.tensor.matmul(ps[:, 0:N], wt["phi_w3"][:], h2[:, g * 512:g * 512 + N])
                encfm = sp.tile([128, 512], bf16, tag="encfm", bufs=2, name="encfm")
                relu_store(encfm[:, 0:N], ps[:, 0:N], wt["phi_b3"][:])
                c0 = g * 4
                nch = min(4, C + 1 - c0)
                nc.sync.dma_start_transpose(out=enc_tok[:, c0:c0 + nch, 0:128],
                                            in_=encfm[:, 0:nch * CH])

            def u_aggdve():
                aggs = sp.tile([128, 1], f32, tag="aggs", bufs=2, name="aggs")
                nc.vector.tensor_reduce(aggs[:], acc[:, 0:NG], mybir.AxisListType.X, Alu.add)
                agg_in = sp.tile([128, 1], bf16, tag="aggin", bufs=2, name="agg_in")
                nc.vector.tensor_scalar(agg_in[:], aggs[:], negc3nr[:, r:r + 1],
                                        recipL1[:, r:r + 1], Alu.add, Alu.mult)
                state[r, "agg_in"] = agg_in

            def u_memset():
                nc.gpsimd.memset(enc_tok[:, 0:C + 1, 128:129], 1.0)

            units = [u_memset]
            units += [lambda g=g: u_l1(g) for g in range(NG)]
            for g in range(NG):
                units.append(lambda g=g: u_psi2(g))
                units.append(lambda g=g: u_phi2(g))
            for g in range(NG):
                units.append(lambda g=g: u_psi3(g))
                units.append(lambda g=g: u_phi3(g))
            units.append(u_aggdve)
            return units

        def phase_agg_pe(r):
            ps_a = pp.tile([128, 512], f32, tag="L2", bufs=3, name="ps_a")
            nc.tensor.matmul(ps_a[:, 0:1], wt["rho_attn_w"][:], state[r, "agg_in"][:])
            agg2 = sp.tile([128, 1], bf16, tag="agg2", bufs=2, name="agg2")
            nc.scalar.activation(agg2[:], ps_a[:, 0:1], Act.Relu, bias=wt["rho_attn_b"][:])
            ps_dot = pp.tile([4, 132], f32, tag="tl", bufs=1, name="ps_dot")
            nc.tensor.matmul(ps_dot[0:1, 0:H], agg2[:], wt["Vagg"][:])
            vxe = sp.tile([NF, H], bf16, tag="vxe", bufs=2, name="vxe")
            nc.vector.tensor_copy(vxe[:], wt["vxe_const"][:])
            nc.vector.tensor_copy(vxe[32:33, :], ps_dot[0:1, 0:H])
            state[r, "vxe"] = vxe

        def tail_units(r):
            """Yield emit closures for row r's attention tail (no relu deps:
            these are the PE filler work interleaved into row r+1's MLP)."""
            C, Tp, Text, NG = row_geom(r)
            xT, vxe = state[r, "xT"], state[r, "vxe"]
            enc_tok = state[r, "enc_tok"]
            ps_pre = pp.tile([128, 132], f32, tag="tl", bufs=1, name="ps_pre")
            e_tok = sp.tile([128, 132], bf16, tag="etok", bufs=2, name="e_tok")
            ps_hh = [None]

            def u_pre(c):
                nc.tensor.matmul(ps_pre[:, c * 4:(c + 1) * 4],
                                 xT[:, c * CH:(c + 1) * CH], vxe[:])

            def u_exp():
                eng_load[0] += 0.4
                nc.scalar.activation(e_tok[:, 0:(C + 1) * 4], ps_pre[:, 0:(C + 1) * 4], Act.Exp)
                ps_hh[0] = pp.tile([4, 132], f32, tag="tl", bufs=1, name="ps_hh")

            def u_hh(c):
                nc.tensor.matmul(ps_hh[0][:, 0:129], e_tok[:, c * 4:(c + 1) * 4],
                                 enc_tok[:, c, 0:129], start=(c == 0), stop=(c == C))

            def u_fin():
                rz = sp.tile([4, 1], f32, tag="rz", bufs=2, name="rz")
                nc.vector.reciprocal(rz[:], ps_hh[0][:, 128:129])
                hh_sb = sp.tile([4, 128], bf16, tag="hhsb", bufs=2, name="hh_sb")
                nc.vector.tensor_scalar(hh_sb[:], ps_hh[0][:, 0:128], rz[:], None, Alu.mult)
                ps_tr = pp.tile([128, 4], bf16, tag="tl", bufs=1, name="ps_tr")
                nc.tensor.transpose(ps_tr[:], hh_sb[:], ident[0:H, 0:H])
                nc.vector.tensor_copy(feat_all[:, r, :], ps_tr[:])
                for k in ("xT", "h1p1", "h2", "enc_tok", "vxe", "agg_in"):
                    state.pop((r, k), None)

            units = [lambda c=c: u_pre(c) for c in range(C + 1)]
            units.append(u_exp)
            units += [lambda c=c: u_hh(c) for c in range(C + 1)]
            units.append(u_fin)
            return units

        # ---- software-pipelined emission: row r's tail chunks interleave as
        # PE filler work between row r+1's relu-paced MLP groups.
        phase_dma(0)
        phase_dma(1)
        for r in range(NSLOTS):
            mu = mlp_units(r)
            tu = tail_units(r - 1) if r >= 1 else []
            ti = 0
            for i, u in enumerate(mu):
                u()
                want = (i + 1) * len(tu) // len(mu)
                while ti < want:
                    tu[ti]()
                    ti += 1
            while ti < len(tu):
                tu[ti]()
                ti += 1
            if r + 2 < NSLOTS:
                phase_dma(r + 2)
            phase_agg_pe(r)
        for u in tail_units(NSLOTS - 1):
            u()

        # ---- rho MLP over all 8 rows ----
        ps_r1 = pp.tile([128, 512], f32, tag="L2", bufs=3, name="ps_r1")
        for h in range(H):
            nc.tensor.matmul(ps_r1[:, 0:NSLOTS], wt["rho_w1"][:, h, :], feat_all[:, :, h],
                             start=(h == 0), stop=(h == H - 1))
        r1 = sp.tile([128, NSLOTS], bf16, tag="r1", name="r1")
        nc.scalar.activation(r1[:], ps_r1[:, 0:NSLOTS], Act.Relu, bias=wt["rho_b1"][:])
        ps_r2 = pp.tile([128, 512], f32, tag="L2", bufs=3, name="ps_r2")
        nc.tensor.matmul(ps_r2[:, 0:NSLOTS], wt["rho_w2"][:], r1[:])
        r2 = sp.tile([128, NSLOTS], bf16, tag="r2", name="r2")
        nc.scalar.activation(r2[:], ps_r2[:, 0:NSLOTS], Act.Relu, bias=wt["rho_b2"][:])
        ps_r3 = pp.tile([4, 132], f32, tag="tl", bufs=1, name="ps_r3")
        nc.tensor.matmul(ps_r3[0:1, 0:NSLOTS], wt["rho_w3"][:], r2[:])
        res = sp.tile([1, NSLOTS], f32, tag="res", name="res")
        nc.scalar.activation(res[:], ps_r3[0:1, 0:NSLOTS], Act.Sigmoid, bias=wt["rho_b3"][:])
        nc.sync.dma_start(out=d_out[:].rearrange("r one -> one r"), in_=res[:])
    return nc


def _patch_tile_drain(tile_mod, mybir):
    """Walrus in this env rejects >1 sync wait per instruction. Two fixes:
    1) split the Tile tail drain's waits across sequential drains;
    2) a post-pass over the final BIR that moves extra waits of ANY
       instruction onto standalone NoOps inserted just before it."""
    from concourse.vector_clock import ScopedClock
    if getattr(tile_mod.TileContext, "_drain_patched", False):
        return

    def _drain_and_barrier(self, tick_clock, wait_clock):
        nc = self.nc
        drain_inst = nc.sync.drain()
        wait_clock.add_sem_waits(drain_inst.ins, ScopedClock({None: tick_clock.global_clock}))
        si = drain_inst.ins.sync_info
        waits = list(si.on_wait or [])
        if len(waits) > 1:
            si.on_wait = waits[:1]
            for i in range(1, len(waits)):
                extra = nc.sync.drain()
                esi = extra.ins.sync_info
                if esi is None:
                    extra.ins.sync_info = mybir.SyncInfo(on_wait=waits[i:i + 1], on_update=[])
                else:
                    esi.on_wait = waits[i:i + 1]
        nc.all_engine_barrier()
        popped = nc._tile_sem_poison_stack.pop()
        assert popped is self._sem_poison
        nc.clear_and_free_semaphores(list(self.sems.allocated().values()))
        nc.all_engine_barrier()

    tile_mod.TileContext._drain_and_barrier = _drain_and_barrier

    _orig_exit = tile_mod.TileContext.__exit__

    def _exit(self, exc_type, exc_val, exc_tb):
        r = _orig_exit(self, exc_type, exc_val, exc_tb)
        if exc_type is None and getattr(tile_mod.TileContext, "_split_waits", True):
            _split_multi_waits(self.nc, mybir)
        return r

    def _split_multi_waits(nc, mybir):
        n = [0]
        for f in nc.m.functions:
            for bb in f.blocks:
                insts = bb.instructions
                out = []
                for inst in insts:
                    si = inst.sync_info
                    waits = list(si.on_wait) if (si and si.on_wait) else []
                    if len(waits) > 1:
                        for w in waits[:-1]:
                            n[0] += 1
                            nop = mybir.InstNoOp(name=f"I-ws-{n[0]}", ins=[], outs=[])
                            nop.engine = inst.engine
                            nop.sync_info = mybir.SyncInfo(on_wait=[w], on_update=[])
                            out.append(nop)
                        si.on_wait = waits[-1:]
                    out.append(inst)
                if len(out) != len(insts):
                    bb.instructions = out

    tile_mod.TileContext.__exit__ = _exit
    tile_mod.TileContext._drain_patched = True


_CACHE = {}
last_results = None


def _maybe_install_ntff_shim():
    """The image's antenv lacks axon_hooks; register the ctypes NTFF hook so
    run_bass_kernel_spmd(trace=True) can profile."""
    import sys, types
    if "antenv.axon_hooks" in sys.modules:
        return
    try:
        from trn_agent_boot.trn_boot import _ntff_profile_via_ctypes
        hook = _ntff_profile_via_ctypes("/opt/axon/libaxon_pjrt.so")
    except Exception:
        hook = None
    mod = types.ModuleType("antenv.axon_hooks")
    mod.get_axon_ntff_profile_hook = lambda: hook
    sys.modules["antenv.axon_hooks"] = mod


def _to_bf16(a):
    import ml_dtypes
    return np.asarray(a, np.float32).astype(ml_dtypes.bfloat16)


def _host_prep(inp):
    """Featurize + fold weights on host. Returns (const_map, xfeat, order, Cs)."""
    times = np.asarray(inp["times"], np.float32)[..., 0]      # [B,T]
    values = np.asarray(inp["values"], np.float32)[..., 0]    # [B,T]
    meas = np.asarray(inp["measurements"], np.int64)          # [B,T]
    lengths = np.asarray(inp["lengths"], np.int64)            # [B]

    ts = (MAXTS ** np.linspace(0.0, 1.0, NPOS // 2)).astype(np.float32)
    scaled = times[:, :, None] / ts[None, None, :]            # [B,T,8]
    xfeat = np.empty((B, NF, T), np.float32)
    xfeat[:, 0:8] = np.sin(scaled).transpose(0, 2, 1)
    xfeat[:, 8:16] = np.cos(scaled).transpose(0, 2, 1)
    xfeat[:, 16] = values
    oh = (meas[:, None, :] == np.arange(NMOD)[None, :, None])
    xfeat[:, 17:32] = oh.astype(np.float32)
    xfeat[:, 32] = 1.0
    xfeat[:, 33] = (np.arange(T)[None, :] >= lengths[:, None]).astype(np.float32)
    xfeat = _to_bf16(xfeat)

    f = lambda k: np.asarray(inp[k], np.float32)
    w1p = np.concatenate([f("phi_w1"), f("phi_b1")[None, :],
                          np.zeros((1, 128), np.float32)], 0)
    w1s = np.concatenate([f("psi_w1"), f("psi_b1")[None, :],
                          np.full((1, 128), -1e9, np.float32)], 0)
    wq = f("W_q") / np.sqrt(DP)
    Wk = f("W_k")
    Vx = np.stack([Wk[0:32, h * DP:(h + 1) * DP] @ wq[h] for h in range(H)], -1)
    Vagg = np.stack([Wk[32:, h * DP:(h + 1) * DP] @ wq[h] for h in range(H)], -1)
    vxe_const = np.zeros((NF, H), np.float32)
    vxe_const[0:32] = Vx
    vxe_const[33] = -1e9

    const_map = {
        "ident": _to_bf16(np.eye(128, dtype=np.float32)),
        "w1p": _to_bf16(w1p), "w1s": _to_bf16(w1s),
        "phi_w2": _to_bf16(f("phi_w2")), "psi_w2": _to_bf16(f("psi_w2")),
        "phi_w3": _to_bf16(f("phi_w3")), "psi_w3": _to_bf16(f("psi_w3")),
        "rho_attn_w": _to_bf16(f("rho_attn_w")), "Vagg": _to_bf16(Vagg),
        "vxe_const": _to_bf16(vxe_const),
        "demo_w1": _to_bf16(f("demo_w1")), "demo_w2": _to_bf16(f("demo_w2")),
        "rho_w1": np.ascontiguousarray(
            _to_bf16(f("rho_w1")).reshape(H, 128, 128).transpose(1, 0, 2)),
        "rho_w2": _to_bf16(f("rho_w2")), "rho_w3": _to_bf16(f("rho_w3")),
        "phi_b2": f("phi_b2"), "psi_b2": f("psi_b2"),
        "phi_b3": f("phi_b3"), "psi_b3": f("psi_b3"),
        "rho_attn_b": f("rho_attn_b"), "demo_b1": f("demo_b1"),
        "demo_b2": f("demo_b2"), "rho_b1": f("rho_b1"),
        "rho_b2": f("rho_b2"), "rho_b3": f("rho_b3"),
    }

    order = np.argsort(-lengths, kind="stable")
    Cs = []
    for s in range(NSLOTS):
        ranks = order[s * NCORES:(s + 1) * NCORES]
        Cs.append(int(np.ceil(lengths[ranks].max() / CH)))
    return const_map, xfeat, order, Cs, lengths


def kernel(**inputs):
    import os
    import concourse.bass as bass
    import concourse.mybir as mybir
    import concourse.tile as tile_mod
    from concourse import bass_utils

    _patch_tile_drain(tile_mod, mybir)

    inp = {k: np.asarray(v) for k, v in inputs.items()}
    const_map, xfeat, order, Cs, lengths = _host_prep(inp)
    demo = _to_bf16(np.asarray(inp["demo"], np.float32))

    key = (tuple(Cs), lengths.tobytes())
    ck = tuple(Cs)
    if ck not in _CACHE:
        _CACHE[ck] = _build_nc(Cs, tile_mod, bass, mybir)
    nc = _CACHE[ck]

    in_maps = []
    for core in range(NCORES):
        rows = [order[s * NCORES + core] for s in range(NSLOTS)]
        lens = lengths[rows].astype(np.int64)
        ninv = np.array([(Cs[s] + 1) * CH - int(lens[s]) - 1
                         for s in range(NSLOTS)], np.float32)
        m = {
            "xfeat_r": np.ascontiguousarray(xfeat[rows]),
            "demo_r": np.ascontiguousarray(demo[rows]),
            "ninv_neg": np.ascontiguousarray(
                np.broadcast_to(-ninv[None, :], (128, NSLOTS))),
            "recipL1": np.ascontiguousarray(np.broadcast_to(
                (1.0 / (lens + 1).astype(np.float32))[None, :], (128, NSLOTS))),
        }
        m.update(const_map)
        in_maps.append(m)

    trace = os.environ.get("KERNEL_TRACE", "0") == "1"
    kw = {}
    if trace:
        _maybe_install_ntff_shim()
        kw = dict(trace=True, tmpdir=os.environ.get("KERNEL_TRACE_DIR") or None)
    res = bass_utils.run_bass_kernel_spmd(nc, in_maps, core_ids=list(range(NCORES)), **kw)
    global last_results
    last_results = res
    out = np.zeros((B, 1), np.float32)
    for core in range(NCORES):
        for s in range(NSLOTS):
            out[order[s * NCORES + core], 0] = res.results[core]["out"][s, 0]
    return out


# revision 11
# speedup vs baseline: 1.3947x; 1.3111x over previous
"""Trainium2 Bass kernel for nn_DeepSetAttentionModel (segment_reduce) — v2.

Strategy (pure data parallel, 8 NeuronCores, host featurization):
- Host sorts the 64 set rows by length (desc); rank k -> core k%8, slot
  k//8. All cores run ONE SPMD program whose per-slot chunk counts are
  ceil(slot-max-length/128), so the program is identical across cores.
- Host precomputes the full 34-feature token matrix in feature-major
  (bf16): rows 0:16 sin/cos positional encodings, 16 value, 17:32
  one-hot, 32 const-1 (bias carrier), 33 invalid-flag. It is DMA'd
  straight into the feature-major xT layout the MLP matmuls consume:
  no on-device featurization, no x transposes, no Act-engine Sin.
- Layer-1 biases are folded into the weights via the const-1 row; the
  phi1/psi1 PSUM banks are adjacent so ONE Act relu covers both.
- psi of invalid tokens is forced to a known constant c3: psi_w1 gets
  an extra -1e9 row against the invalid flag (p1=0), and the resulting
  constant bias chain c3 = relu(psi_w3^T relu(psi_b2) + psi_b3) is
  removed with a host-known per-row count correction.
- preattn is computed token-major per 128-chunk ([34,128] stationary
  slices of xT against a [34,4] folded key vector whose row 32 carries
  the agg contribution and row 33 carries -1e9 for invalid tokens).
  |preattn| < 1 for this model, so exp needs no max-subtraction;
  invalid tokens underflow to exactly 0 and drop out of softmax sums.
- enc (phi output) is computed feature-major and moved token-major via
  xbar DMA transposes; the per-chunk weighted head sum contracts a
  129th all-ones column to produce the softmax normalizer for free.
"""
import numpy as np

B, T = 64, 4096
CH = 128
NPOS, V, NMOD = 16, 1, 15
NF = 34                      # 16 tt + 1 value + 15 onehot + 1 const + 1 inv
DP, H = 64, 4
MAXTS = 100.0
NCORES = 8
NSLOTS = B // NCORES


def _build_nc(Cs, tile_mod, bass, mybir):
    """Build the SPMD program for per-slot chunk counts Cs (len 8)."""
    f32 = mybir.dt.float32
    bf16 = mybir.dt.bfloat16
    Alu = mybir.AluOpType
    Act = mybir.ActivationFunctionType
    Cmax = max(Cs)
    NGmax = (((Cmax + 1) * CH) + 511) // 512

    nc = bass.Bass()
    dt_in = {}

    def din(name, shape, dtype=f32):
        dt_in[name] = nc.dram_tensor(name, list(shape), dtype, kind="ExternalInput")
        return dt_in[name]

    # per-core inputs
    d_xfeat = din("xfeat_r", [NSLOTS, NF, T], bf16)
    d_demo = din("demo_r", [NSLOTS, 8], bf16)
    din("ninv_neg", [128, NSLOTS])
    din("recipL1", [128, NSLOTS])
    # replicated constants / weights
    din("ident", [128, 128], bf16)
    WB = [("w1p", [NF, 128], bf16), ("w1s", [NF, 128], bf16),
          ("phi_w2", [128, 128], bf16), ("psi_w2", [128, 128], bf16),
          ("phi_w3", [128, 128], bf16), ("psi_w3", [128, 128], bf16),
          ("rho_attn_w", [128, 128], bf16), ("Vagg", [128, H], bf16),
          ("vxe_const", [NF, H], bf16),
          ("demo_w1", [8, 128], bf16), ("demo_w2", [128, 32], bf16),
          ("rho_w1", [128, H, 128], bf16), ("rho_w2", [128, 128], bf16),
          ("rho_w3", [128, 1], bf16)]
    for nm, shp, dt_ in WB:
        din(nm, shp, dt_)
    BIASES = [("phi_b2", 128), ("psi_b2", 128), ("phi_b3", 128), ("psi_b3", 128),
              ("rho_attn_b", 128), ("demo_b1", 128), ("demo_b2", 32),
              ("rho_b1", 128), ("rho_b2", 128), ("rho_b3", 1)]
    for nm, n in BIASES:
        din(nm, [n])
    d_out = nc.dram_tensor("out", [NSLOTS, 1], f32, kind="ExternalOutput")

    from contextlib import ExitStack
    with tile_mod.TileContext(nc) as tc, ExitStack() as stack:
        cp = stack.enter_context(tc.tile_pool(name="const", bufs=1))
        sp = stack.enter_context(tc.tile_pool(name="sbuf", bufs=1))
        pp = stack.enter_context(tc.tile_pool(name="psum", bufs=1, space="PSUM"))

        # ---- load constants / weights ----
        def load(name, shape, dtype=f32, ap=None):
            t = cp.tile(shape, dtype, tag=name, name=name)
            nc.sync.dma_start(out=t[:], in_=ap if ap is not None else dt_in[name][:])
            return t

        wt = {}
        for nm, shp, dt_ in WB:
            wt[nm] = load(nm, shp, dt_)
        for nm, n in BIASES:
            wt[nm] = load(nm, [n, 1], ap=dt_in[nm][:].unsqueeze(1))
        ident = load("ident", [128, 128], bf16)
        ninv_neg = load("ninv_neg", [128, NSLOTS])
        recipL1 = load("recipL1", [128, NSLOTS])
        demoT = load("demoT", [8, NSLOTS], bf16, ap=d_demo[:].rearrange("r f -> f r"))

        # ---- setup: demo encoder for all 8 slots -> demo_encT [34, 8] ----
        ps_d = pp.tile([128, 512], f32, tag="mm", bufs=7, name="ps_d")
        nc.tensor.matmul(ps_d[:, 0:NSLOTS], wt["demo_w1"][:], demoT[:])
        dh1 = sp.tile([128, NSLOTS], bf16, tag="dh1", name="dh1")
        nc.scalar.activation(dh1[:], ps_d[:, 0:NSLOTS], Act.Relu, bias=wt["demo_b1"][:])
        ps_d2 = pp.tile([128, 512], f32, tag="mm", bufs=7, name="ps_d2")
        nc.tensor.matmul(ps_d2[0:32, 0:NSLOTS], wt["demo_w2"][:], dh1[:])
        demo_encT = cp.tile([NF, NSLOTS], bf16, tag="demo_encT", name="demo_encT")
        nc.scalar.activation(demo_encT[0:32, :], ps_d2[0:32, 0:NSLOTS],
                             Act.Identity, bias=wt["demo_b2"][:])
        nc.gpsimd.memset(demo_encT[32:34, :], 0.0)
        nc.gpsimd.memset(demo_encT[32:33, :], 1.0)

        # ---- setup: c3 = relu(psi_w3^T relu(psi_b2) + psi_b3), correction ----
        p2c = sp.tile([128, 1], bf16, tag="p2c", name="p2c")
        nc.scalar.activation(p2c[:], wt["psi_b2"][:], Act.Relu)
        ps_c3 = pp.tile([128, 512], f32, tag="mm", bufs=7, name="ps_c3")
        nc.tensor.matmul(ps_c3[:, 0:1], wt["psi_w3"][:], p2c[:])
        c3 = sp.tile([128, 1], f32, tag="c3", name="c3")
        nc.scalar.activation(c3[:], ps_c3[:, 0:1], Act.Relu, bias=wt["psi_b3"][:])
        negc3nr = cp.tile([128, NSLOTS], f32, tag="negc3nr", name="negc3nr")
        nc.vector.tensor_tensor(out=negc3nr[:], in0=c3[:].to_broadcast([128, NSLOTS]),
                                in1=ninv_neg[:], op=Alu.mult)

        feat_all = sp.tile([128, NSLOTS, H], bf16, tag="feat_all", name="feat_all")

        # ---- per-row phases (emitted software-pipelined below) ----
        state = {}
        eng_load = [0.0, 0.0]  # running Act / DVE load estimate (us)

        def relu_store(out_ap, in_ap, bias_ap, accum_ap=None):
            """PSUM->SBUF relu on whichever of Act/DVE is less loaded
            (costs are HW-measured per 512-col group)."""
            ca, cd = (1.21, 0.95) if accum_ap is not None else (0.85, 0.90)
            if eng_load[0] + ca <= eng_load[1] + cd:
                eng_load[0] += ca
                nc.scalar.activation(out_ap, in_ap, Act.Relu,
                                     bias=0.0 if bias_ap is None else bias_ap,
                                     accum_out=accum_ap)
            else:
                eng_load[1] += cd
                if bias_ap is None:
                    nc.vector.tensor_scalar(out_ap, in_ap, 0.0, None,
                                            Alu.max, accum_out=accum_ap)
                else:
                    nc.vector.tensor_scalar(out_ap, in_ap, bias_ap, 0.0,
                                            Alu.add, Alu.max, accum_out=accum_ap)

        def row_geom(r):
            C = Cs[r]
            return C, C * CH, (C + 1) * CH, ((C + 1) * CH + 511) // 512

        def phase_dma(r):
            C, Tp, Text, NG = row_geom(r)
            xT = sp.tile([NF, (Cmax + 1) * CH], bf16, tag="xT", bufs=3, name="xT")
            nc.sync.dma_start(out=xT[:, 0:Tp], in_=d_xfeat[r, :, 0:Tp])
            nc.gpsimd.memset(xT[:, Tp + 1:Text], 0.0)
            # rows 32/33 (const, inv) both 1.0 in the padding: the -1e9 inv
            # weight dominates, so a stray aggdot via the const row is inert
            nc.gpsimd.memset(xT[32:34, Tp + 1:Text], 1.0)
            nc.gpsimd.tensor_copy(xT[:, Tp:Tp + 1], demo_encT[:, r:r + 1])
            state[r, "xT"] = xT

        def mlp_units(r):
            """Yield per-group emit closures for row r's MLP, interleaved so
            consecutive PE matmuls feed alternating relu consumers."""
            C, Tp, Text, NG = row_geom(r)
            xT = state[r, "xT"]
            h1p1 = sp.tile([128, NGmax * 1024], bf16, tag="h1p1", bufs=2, name="h1p1")
            h2 = sp.tile([128, (Cmax + 1) * CH], bf16, tag="h2", bufs=2, name="h2")
            p2 = sp.tile([128, (Cmax + 1) * CH], bf16, tag="p2", bufs=2, name="p2")
            enc_tok = sp.tile([128, Cmax + 1, 144], bf16, tag="enctok", bufs=2, name="enc_tok")
            acc = sp.tile([128, 16], f32, tag="acc", bufs=2, name="acc")
            psi3s = sp.tile([128, 512], bf16, tag="psi3s", bufs=2, name="psi3s")
            state[r, "enc_tok"] = enc_tok

            def u_l1h(g):
                N = min(512, Text - g * 512)
                ps = pp.tile([128, 512], f32, tag="mm", bufs=7, name="ps_l1h")
                nc.tensor.matmul(ps[:, 0:N], wt["w1p"][:], xT[:, g * 512:g * 512 + N])
                relu_store(h1p1[:, g * 1024:g * 1024 + N], ps[:, 0:N], None)

            def u_l1p(g):
                N = min(512, Text - g * 512)
                ps = pp.tile([128, 512], f32, tag="mm", bufs=7, name="ps_l1p")
                nc.tensor.matmul(ps[:, 0:N], wt["w1s"][:], xT[:, g * 512:g * 512 + N])
                relu_store(h1p1[:, g * 1024 + 512:g * 1024 + 512 + N], ps[:, 0:N], None)

            def u_psi2(g):
                N = min(512, Text - g * 512)
                ps = pp.tile([128, 512], f32, tag="mm", bufs=7, name="ps_psi2")
                nc.tensor.matmul(ps[:, 0:N], wt["psi_w2"][:],
                                 h1p1[:, g * 1024 + 512:g * 1024 + 512 + N])
                relu_store(p2[:, g * 512:g * 512 + N], ps[:, 0:N], wt["psi_b2"][:])

            def u_phi2(g):
                N = min(512, Text - g * 512)
                ps = pp.tile([128, 512], f32, tag="mm", bufs=7, name="ps_phi2")
                nc.tensor.matmul(ps[:, 0:N], wt["phi_w2"][:], h1p1[:, g * 1024:g * 1024 + N])
                relu_store(h2[:, g * 512:g * 512 + N], ps[:, 0:N], wt["phi_b2"][:])

            def u_psi3(g):
                N = min(512, Text - g * 512)
                ps = pp.tile([128, 512], f32, tag="mm", bufs=7, name="ps_psi3")
                nc.tensor.matmul(ps[:, 0:N], wt["psi_w3"][:], p2[:, g * 512:g * 512 + N])
                relu_store(psi3s[:, 0:N], ps[:, 0:N], wt["psi_b3"][:],
                           accum_ap=acc[:, g:g + 1])

            def u_phi3(g):
                N = min(512, Text - g * 512)
                ps = pp.tile([128, 512], f32, tag="mm", bufs=7, name="ps_phi3")
                nc.tensor.matmul(ps[:, 0:N], wt["phi_w3"][:], h2[:, g * 512:g * 512 + N])
                encfm = sp.tile([128, 512], bf16, tag="encfm", bufs=2, name="encfm")
                relu_store(encfm[:, 0:N], ps[:, 0:N], wt["phi_b3"][:])
                c0 = g * 4
                nch = min(4, C + 1 - c0)
                nc.sync.dma_start_transpose(out=enc_tok[:, c0:c0 + nch, 0:128],
                                            in_=encfm[:, 0:nch * CH])

            def u_aggdve():
                aggs = sp.tile([128, 1], f32, tag="aggs", bufs=2, name="aggs")
                nc.vector.tensor_reduce(aggs[:], acc[:, 0:NG], mybir.AxisListType.X, Alu.add)
                agg_in = sp.tile([128, 1], bf16, tag="aggin", bufs=2, name="agg_in")
                nc.vector.tensor_scalar(agg_in[:], aggs[:], negc3nr[:, r:r + 1],
                                        recipL1[:, r:r + 1], Alu.add, Alu.mult)
                state[r, "agg_in"] = agg_in

            def u_memset():
                nc.gpsimd.memset(enc_tok[:, 0:C + 1, 128:129], 1.0)

            units = [u_memset]
            for g in range(NG):
                units.append(lambda g=g: u_l1h(g))
                units.append(lambda g=g: u_l1p(g))
            for g in range(NG):
                units.append(lambda g=g: u_psi2(g))
                units.append(lambda g=g: u_phi2(g))
            for g in range(NG):
                units.append(lambda g=g: u_psi3(g))
                units.append(lambda g=g: u_phi3(g))
            units.append(u_aggdve)
            return units

        def phase_agg_pe(r):
            ps_a = pp.tile([128, 512], f32, tag="mm", bufs=7, name="ps_a")
            nc.tensor.matmul(ps_a[:, 0:1], wt["rho_attn_w"][:], state[r, "agg_in"][:])
            agg2 = sp.tile([128, 1], bf16, tag="agg2", bufs=2, name="agg2")
            nc.scalar.activation(agg2[:], ps_a[:, 0:1], Act.Relu, bias=wt["rho_attn_b"][:])
            ps_dot = pp.tile([4, 132], f32, tag="tl", bufs=1, name="ps_dot")
            nc.tensor.matmul(ps_dot[0:1, 0:H], agg2[:], wt["Vagg"][:])
            vxe = sp.tile([NF, H], bf16, tag="vxe", bufs=2, name="vxe")
            nc.gpsimd.tensor_copy(vxe[:], wt["vxe_const"][:])
            nc.vector.tensor_copy(vxe[32:33, :], ps_dot[0:1, 0:H])
            state[r, "vxe"] = vxe

        def tail_units(r):
            """Yield emit closures for row r's attention tail (no relu deps:
            these are the PE filler work interleaved into row r+1's MLP)."""
            C, Tp, Text, NG = row_geom(r)
            xT, vxe = state[r, "xT"], state[r, "vxe"]
            enc_tok = state[r, "enc_tok"]
            ps_pre = pp.tile([128, 132], f32, tag="tl", bufs=1, name="ps_pre")
            e_tok = sp.tile([128, 132], bf16, tag="etok", bufs=2, name="e_tok")
            ps_hh = [None]

            def u_pre(c):
                nc.tensor.matmul(ps_pre[:, c * 4:(c + 1) * 4],
                                 xT[:, c * CH:(c + 1) * CH], vxe[:])

            def u_exp():
                eng_load[0] += 0.4
                nc.scalar.activation(e_tok[:, 0:(C + 1) * 4], ps_pre[:, 0:(C + 1) * 4], Act.Exp)
                ps_hh[0] = pp.tile([4, 132], f32, tag="tl", bufs=1, name="ps_hh")

            def u_hh(c):
                nc.tensor.matmul(ps_hh[0][:, 0:129], e_tok[:, c * 4:(c + 1) * 4],
                                 enc_tok[:, c, 0:129], start=(c == 0), stop=(c == C))

            def u_fin():
                rz = sp.tile([4, 1], f32, tag="rz", bufs=2, name="rz")
                nc.vector.reciprocal(rz[:], ps_hh[0][:, 128:129])
                hh_sb = sp.tile([4, 128], bf16, tag="hhsb", bufs=2, name="hh_sb")
                nc.vector.tensor_scalar(hh_sb[:], ps_hh[0][:, 0:128], rz[:], None, Alu.mult)
                ps_tr = pp.tile([128, 4], bf16, tag="tl", bufs=1, name="ps_tr")
                nc.tensor.transpose(ps_tr[:], hh_sb[:], ident[0:H, 0:H])
                nc.vector.tensor_copy(feat_all[:, r, :], ps_tr[:])
                for k in ("xT", "h1p1", "h2", "enc_tok", "vxe", "agg_in"):
                    state.pop((r, k), None)

            units = [lambda c=c: u_pre(c) for c in range(C + 1)]
            units.append(u_exp)
            units += [lambda c=c: u_hh(c) for c in range(C + 1)]
            units.append(u_fin)
            return units

        # ---- software-pipelined emission: row r's tail chunks interleave as
        # PE filler work between row r+1's relu-paced MLP groups.
        phase_dma(0)
        phase_dma(1)
        for r in range(NSLOTS):
            mu = mlp_units(r)
            tu = tail_units(r - 1) if r >= 1 else []
            ti = 0
            for i, u in enumerate(mu):
                u()
                want = (i + 1) * len(tu) // len(mu)
                while ti < want:
                    tu[ti]()
                    ti += 1
            while ti < len(tu):
                tu[ti]()
                ti += 1
            if r + 2 < NSLOTS:
                phase_dma(r + 2)
            phase_agg_pe(r)
        for u in tail_units(NSLOTS - 1):
            u()

        # ---- rho MLP over all 8 rows ----
        ps_r1 = pp.tile([128, 512], f32, tag="mm", bufs=7, name="ps_r1")
        for h in range(H):
            nc.tensor.matmul(ps_r1[:, 0:NSLOTS], wt["rho_w1"][:, h, :], feat_all[:, :, h],
                             start=(h == 0), stop=(h == H - 1))
        r1 = sp.tile([128, NSLOTS], bf16, tag="r1", name="r1")
        nc.scalar.activation(r1[:], ps_r1[:, 0:NSLOTS], Act.Relu, bias=wt["rho_b1"][:])
        ps_r2 = pp.tile([128, 512], f32, tag="mm", bufs=7, name="ps_r2")
        nc.tensor.matmul(ps_r2[:, 0:NSLOTS], wt["rho_w2"][:], r1[:])
        r2 = sp.tile([128, NSLOTS], bf16, tag="r2", name="r2")
        nc.scalar.activation(r2[:], ps_r2[:, 0:NSLOTS], Act.Relu, bias=wt["rho_b2"][:])
        ps_r3 = pp.tile([4, 132], f32, tag="tl", bufs=1, name="ps_r3")
        nc.tensor.matmul(ps_r3[0:1, 0:NSLOTS], wt["rho_w3"][:], r2[:])
        res = sp.tile([1, NSLOTS], f32, tag="res", name="res")
        nc.scalar.activation(res[:], ps_r3[0:1, 0:NSLOTS], Act.Sigmoid, bias=wt["rho_b3"][:])
        nc.sync.dma_start(out=d_out[:].rearrange("r one -> one r"), in_=res[:])
    return nc


def _patch_tile_drain(tile_mod, mybir):
    """Walrus in this env rejects >1 sync wait per instruction. Two fixes:
    1) split the Tile tail drain's waits across sequential drains;
    2) a post-pass over the final BIR that moves extra waits of ANY
       instruction onto standalone NoOps inserted just before it."""
    from concourse.vector_clock import ScopedClock
    if getattr(tile_mod.TileContext, "_drain_patched", False):
        return

    def _drain_and_barrier(self, tick_clock, wait_clock):
        nc = self.nc
        drain_inst = nc.sync.drain()
        wait_clock.add_sem_waits(drain_inst.ins, ScopedClock({None: tick_clock.global_clock}))
        si = drain_inst.ins.sync_info
        waits = list(si.on_wait or [])
        if len(waits) > 1:
            si.on_wait = waits[:1]
            for i in range(1, len(waits)):
                extra = nc.sync.drain()
                esi = extra.ins.sync_info
                if esi is None:
                    extra.ins.sync_info = mybir.SyncInfo(on_wait=waits[i:i + 1], on_update=[])
                else:
                    esi.on_wait = waits[i:i + 1]
        nc.all_engine_barrier()
        popped = nc._tile_sem_poison_stack.pop()
        assert popped is self._sem_poison
        nc.clear_and_free_semaphores(list(self.sems.allocated().values()))
        nc.all_engine_barrier()

    tile_mod.TileContext._drain_and_barrier = _drain_and_barrier

    _orig_exit = tile_mod.TileContext.__exit__

    def _exit(self, exc_type, exc_val, exc_tb):
        r = _orig_exit(self, exc_type, exc_val, exc_tb)
        if exc_type is None and getattr(tile_mod.TileContext, "_split_waits", True):
            _split_multi_waits(self.nc, mybir)
        return r

    def _split_multi_waits(nc, mybir):
        n = [0]
        for f in nc.m.functions:
            for bb in f.blocks:
                insts = bb.instructions
                out = []
                for inst in insts:
                    si = inst.sync_info
                    waits = list(si.on_wait) if (si and si.on_wait) else []
                    if len(waits) > 1:
                        for w in waits[:-1]:
                            n[0] += 1
                            nop = mybir.InstNoOp(name=f"I-ws-{n[0]}", ins=[], outs=[])
                            nop.engine = inst.engine
                            nop.sync_info = mybir.SyncInfo(on_wait=[w], on_update=[])
                            out.append(nop)
                        si.on_wait = waits[-1:]
                    out.append(inst)
                if len(out) != len(insts):
                    bb.instructions = out

    tile_mod.TileContext.__exit__ = _exit
    tile_mod.TileContext._drain_patched = True


_CACHE = {}
last_results = None


def _maybe_install_ntff_shim():
    """The image's antenv lacks axon_hooks; register the ctypes NTFF hook so
    run_bass_kernel_spmd(trace=True) can profile."""
    import sys, types
    if "antenv.axon_hooks" in sys.modules:
        return
    try:
        from trn_agent_boot.trn_boot import _ntff_profile_via_ctypes
        hook = _ntff_profile_via_ctypes("/opt/axon/libaxon_pjrt.so")
    except Exception:
        hook = None
    mod = types.ModuleType("antenv.axon_hooks")
    mod.get_axon_ntff_profile_hook = lambda: hook
    sys.modules["antenv.axon_hooks"] = mod


def _to_bf16(a):
    import ml_dtypes
    return np.asarray(a, np.float32).astype(ml_dtypes.bfloat16)


def _host_prep(inp):
    """Featurize + fold weights on host. Returns (const_map, xfeat, order, Cs)."""
    times = np.asarray(inp["times"], np.float32)[..., 0]      # [B,T]
    values = np.asarray(inp["values"], np.float32)[..., 0]    # [B,T]
    meas = np.asarray(inp["measurements"], np.int64)          # [B,T]
    lengths = np.asarray(inp["lengths"], np.int64)            # [B]

    ts = (MAXTS ** np.linspace(0.0, 1.0, NPOS // 2)).astype(np.float32)
    scaled = times[:, :, None] / ts[None, None, :]            # [B,T,8]
    xfeat = np.empty((B, NF, T), np.float32)
    xfeat[:, 0:8] = np.sin(scaled).transpose(0, 2, 1)
    xfeat[:, 8:16] = np.cos(scaled).transpose(0, 2, 1)
    xfeat[:, 16] = values
    oh = (meas[:, None, :] == np.arange(NMOD)[None, :, None])
    xfeat[:, 17:32] = oh.astype(np.float32)
    xfeat[:, 32] = 1.0
    xfeat[:, 33] = (np.arange(T)[None, :] >= lengths[:, None]).astype(np.float32)
    xfeat = _to_bf16(xfeat)

    f = lambda k: np.asarray(inp[k], np.float32)
    w1p = np.concatenate([f("phi_w1"), f("phi_b1")[None, :],
                          np.zeros((1, 128), np.float32)], 0)
    w1s = np.concatenate([f("psi_w1"), f("psi_b1")[None, :],
                          np.full((1, 128), -1e9, np.float32)], 0)
    wq = f("W_q") / np.sqrt(DP)
    Wk = f("W_k")
    Vx = np.stack([Wk[0:32, h * DP:(h + 1) * DP] @ wq[h] for h in range(H)], -1)
    Vagg = np.stack([Wk[32:, h * DP:(h + 1) * DP] @ wq[h] for h in range(H)], -1)
    vxe_const = np.zeros((NF, H), np.float32)
    vxe_const[0:32] = Vx
    vxe_const[33] = -1e9

    const_map = {
        "ident": _to_bf16(np.eye(128, dtype=np.float32)),
        "w1p": _to_bf16(w1p), "w1s": _to_bf16(w1s),
        "phi_w2": _to_bf16(f("phi_w2")), "psi_w2": _to_bf16(f("psi_w2")),
        "phi_w3": _to_bf16(f("phi_w3")), "psi_w3": _to_bf16(f("psi_w3")),
        "rho_attn_w": _to_bf16(f("rho_attn_w")), "Vagg": _to_bf16(Vagg),
        "vxe_const": _to_bf16(vxe_const),
        "demo_w1": _to_bf16(f("demo_w1")), "demo_w2": _to_bf16(f("demo_w2")),
        "rho_w1": np.ascontiguousarray(
            _to_bf16(f("rho_w1")).reshape(H, 128, 128).transpose(1, 0, 2)),
        "rho_w2": _to_bf16(f("rho_w2")), "rho_w3": _to_bf16(f("rho_w3")),
        "phi_b2": f("phi_b2"), "psi_b2": f("psi_b2"),
        "phi_b3": f("phi_b3"), "psi_b3": f("psi_b3"),
        "rho_attn_b": f("rho_attn_b"), "demo_b1": f("demo_b1"),
        "demo_b2": f("demo_b2"), "rho_b1": f("rho_b1"),
        "rho_b2": f("rho_b2"), "rho_b3": f("rho_b3"),
    }

    order = np.argsort(-lengths, kind="stable")
    Cs = []
    for s in range(NSLOTS):
        ranks = order[s * NCORES:(s + 1) * NCORES]
        Cs.append(int(np.ceil(lengths[ranks].max() / CH)))
    return const_map, xfeat, order, Cs, lengths


def kernel(**inputs):
    import os
    import concourse.bass as bass
    import concourse.mybir as mybir
    import concourse.tile as tile_mod
    from concourse import bass_utils

    _patch_tile_drain(tile_mod, mybir)

    inp = {k: np.asarray(v) for k, v in inputs.items()}
    const_map, xfeat, order, Cs, lengths = _host_prep(inp)
    demo = _to_bf16(np.asarray(inp["demo"], np.float32))

    key = (tuple(Cs), lengths.tobytes())
    ck = tuple(Cs)
    if ck not in _CACHE:
        _CACHE[ck] = _build_nc(Cs, tile_mod, bass, mybir)
    nc = _CACHE[ck]

    in_maps = []
    for core in range(NCORES):
        rows = [order[s * NCORES + core] for s in range(NSLOTS)]
        lens = lengths[rows].astype(np.int64)
        ninv = np.array([(Cs[s] + 1) * CH - int(lens[s]) - 1
                         for s in range(NSLOTS)], np.float32)
        m = {
            "xfeat_r": np.ascontiguousarray(xfeat[rows]),
            "demo_r": np.ascontiguousarray(demo[rows]),
            "ninv_neg": np.ascontiguousarray(
                np.broadcast_to(-ninv[None, :], (128, NSLOTS))),
            "recipL1": np.ascontiguousarray(np.broadcast_to(
                (1.0 / (lens + 1).astype(np.float32))[None, :], (128, NSLOTS))),
        }
        m.update(const_map)
        in_maps.append(m)

    trace = os.environ.get("KERNEL_TRACE", "0") == "1"
    kw = {}
    if trace:
        _maybe_install_ntff_shim()
        kw = dict(trace=True, tmpdir=os.environ.get("KERNEL_TRACE_DIR") or None)
    res = bass_utils.run_bass_kernel_spmd(nc, in_maps, core_ids=list(range(NCORES)), **kw)
    global last_results
    last_results = res
    out = np.zeros((B, 1), np.float32)
    for core in range(NCORES):
        for s in range(NSLOTS):
            out[order[s * NCORES + core], 0] = res.results[core]["out"][s, 0]
    return out
